# revision 23
# baseline (speedup 1.0000x reference)
"""Trainium2 Bass kernel for complex-valued spatial-reduction attention.

x: [B=4, N=2304, C=512] complex64 (re/im f32 planes), H=W=48, 8 heads,
head_dim 64, sr_ratio 2 -> Nk=576.

Sharding: 8 cores = 4 batches x 2 head-groups (4 heads each). Each core:
sr-conv over full C, complex LayerNorm, q/k/v for its heads,
softmax(|q.k^T|) attention, attn @ v, partial output projection.
Host sums the two partials per batch and adds bproj.

v2 structure: scores use K=128 packing (kA=[kr;-ki], kB=[ki;kr],
qcat=[qr;qi]); q-projection and output projection run inside the
attention loop (q/attn-out never round-trip DRAM); softmax runs
Square(ACT f32) + sim^2/add (DVE f32) + batched Ln/Exp runs;
denominator reciprocal on DVE (no act-table thrash); v/attn/proj in f16.
"""

import os
import contextlib

import numpy as np
import ml_dtypes

import concourse.bass as bass
import concourse.mybir as mybir
import concourse.tile as tile
from concourse import bacc
from concourse.masks import make_identity

BF16 = mybir.dt.bfloat16
F16 = mybir.dt.float16
F32 = mybir.dt.float32
F32R = mybir.dt.float32r
AF = mybir.ActivationFunctionType
ALU = mybir.AluOpType

B, N, C, HEADS, HD, SR = 4, 2304, 512, 8, 64, 2
NK = 576
HR = 24
EPS = 1e-5
SCALE = HD ** -0.5  # folded into Wk host-side

M_GROUPS = [
    [(0, 5), (5, 5), (10, 5)],
    [(15, 5), (20, 4)],
]
K_CHUNKS = [(0, 128), (128, 128), (256, 128), (384, 128), (512, 64)]
Q_CHUNKS = [(0, 512), (512, 512), (1024, 512), (1536, 512), (2048, 256)]

# 0 = ln/exp sqrt (safe), 1 = ACT Sqrt LUT (faster; precision probe)
SQRTMODE = int(os.environ.get("KBUILD_SQRT", "1"))
# 0 = DVE tensor_mul(sim, sim) reading same PSUM AP twice; 1 = ACT Square
SQMODE = int(os.environ.get("KBUILD_SQ", "0"))
DEBUG = bool(int(os.environ.get("KBUILD_DEBUG", "0")))
# 1 = baseline ln/exp reciprocal instead of reciprocal_approx_fast
RECIPMODE = int(os.environ.get("KBUILD_RECIP", "0"))


def _r(ap):
    return ap.bitcast(F32R)


def build_nc():
    nc = bacc.Bacc("TRN2", target_bir_lowering=False, debug=False, num_devices=8)

    xT_d = nc.dram_tensor("xT", [2, C, N], F32R, kind="ExternalInput")
    xP_d = nc.dram_tensor("xP", [2, 4 * C, NK], F32R, kind="ExternalInput")
    wc_d = nc.dram_tensor("wc", [3, 4 * C, C], F32R, kind="ExternalInput")
    srb_d = nc.dram_tensor("srb", [2, C], F32R, kind="ExternalInput")
    ones_d = nc.dram_tensor("ones", [1, 512], F32R, kind="ExternalInput")
    wq_d = nc.dram_tensor("wq", [3, C, 256], F32R, kind="ExternalInput")
    wk_d = nc.dram_tensor("wk", [3, C, 256], F32R, kind="ExternalInput")
    wv_d = nc.dram_tensor("wv", [3, C, 256], F32R, kind="ExternalInput")
    wpab_d = nc.dram_tensor("wpab", [2, 4, 128, C], F16, kind="ExternalInput")
    bkv_d = nc.dram_tensor("bkv", [2, 2, 256], F32R, kind="ExternalInput")
    outT_d = nc.dram_tensor("outT", [2, C, N], F16, kind="ExternalOutput")
    dbg = {}
    if DEBUG:
        dbg["kA"] = nc.dram_tensor("dbg_kA", [128, 4, NK], F32, kind="ExternalOutput")
        dbg["kB"] = nc.dram_tensor("dbg_kB", [128, 4, NK], F32, kind="ExternalOutput")
        dbg["qc"] = nc.dram_tensor("dbg_qc", [4, 128, 512], F32, kind="ExternalOutput")
        dbg["s"] = nc.dram_tensor("dbg_s", [2, 128, 2, 512], F32, kind="ExternalOutput")
        dbg["xnT"] = nc.dram_tensor("dbg_xnT", [2, C, NK], F32, kind="ExternalOutput")

    with tile.TileContext(nc) as tc:
        _body(nc, tc, xT_d, xP_d, wc_d, srb_d, ones_d, wq_d, wk_d, wv_d,
              wpab_d, bkv_d, outT_d, dbg)

    nc.compile()
    return nc


def _ln_chunk(nc, work, stats, cre, cim, sz):
    """Complex LayerNorm for one [sz, C] chunk in PSUM -> (xnr, xni)."""
    inv_c = 1.0 / C
    re_sb = work.tile([128, C], F32, tag="ln_re")
    im_sb = work.tile([128, C], F32, tag="ln_im")
    sum_r = stats.tile([128, 1], F32, tag="sum_r")
    sum_i = stats.tile([128, 1], F32, tag="sum_i")
    nc.vector.tensor_copy(re_sb[:sz], cre[:sz, :])
    nc.vector.tensor_copy(im_sb[:sz], cim[:sz, :])
    nc.vector.tensor_reduce(sum_r[:sz], re_sb[:sz], mybir.AxisListType.X, ALU.add)
    nc.vector.tensor_reduce(sum_i[:sz], im_sb[:sz], mybir.AxisListType.X, ALU.add)
    junk = work.tile([128, C], F32, tag="ln_junk", bufs=1)
    sxx = stats.tile([128, 1], F32, tag="sxx")
    sii = stats.tile([128, 1], F32, tag="sii")
    sxi = stats.tile([128, 1], F32, tag="sxi")
    nc.vector.tensor_mul(junk[:sz], re_sb[:sz], re_sb[:sz])
    nc.vector.tensor_reduce(sxx[:sz], junk[:sz], mybir.AxisListType.X, ALU.add)
    nc.vector.tensor_mul(junk[:sz], im_sb[:sz], im_sb[:sz])
    nc.vector.tensor_reduce(sii[:sz], junk[:sz], mybir.AxisListType.X, ALU.add)
    nc.vector.tensor_mul(junk[:sz], re_sb[:sz], im_sb[:sz])
    nc.vector.tensor_reduce(sxi[:sz], junk[:sz], mybir.AxisListType.X, ALU.add)
    mr = stats.tile([128, 1], F32, tag="mr")
    mi = stats.tile([128, 1], F32, tag="mi")
    nc.vector.tensor_scalar_mul(mr[:sz], sum_r[:sz], inv_c)
    nc.vector.tensor_scalar_mul(mi[:sz], sum_i[:sz], inv_c)
    vre = stats.tile([128, 1], F32, tag="vre")
    vim = stats.tile([128, 1], F32, tag="vim")
    tA = stats.tile([128, 1], F32, tag="tA")
    tB = stats.tile([128, 1], F32, tag="tB")
    nc.vector.tensor_sub(tA[:sz], sxx[:sz], sii[:sz])
    nc.vector.tensor_scalar_mul(tA[:sz], tA[:sz], inv_c)
    nc.vector.tensor_mul(vre[:sz], mr[:sz], mr[:sz])
    nc.vector.tensor_mul(tB[:sz], mi[:sz], mi[:sz])
    nc.vector.tensor_sub(vre[:sz], vre[:sz], tB[:sz])
    nc.vector.tensor_sub(vre[:sz], tA[:sz], vre[:sz])
    nc.vector.tensor_scalar_add(vre[:sz], vre[:sz], EPS)
    nc.vector.tensor_mul(tB[:sz], mr[:sz], mi[:sz])
    nc.vector.tensor_scalar_mul(tB[:sz], tB[:sz], 2.0)
    nc.vector.tensor_scalar_mul(vim[:sz], sxi[:sz], 2.0 * inv_c)
    nc.vector.tensor_sub(vim[:sz], vim[:sz], tB[:sz])
    r2 = stats.tile([128, 1], F32, tag="r2")
    nc.vector.tensor_mul(r2[:sz], vre[:sz], vre[:sz])
    nc.vector.tensor_mul(tB[:sz], vim[:sz], vim[:sz])
    nc.vector.tensor_add(r2[:sz], r2[:sz], tB[:sz])

    def _sqrt_newton(out, x, sc):
        # y0 = LUT sqrt(sc*x); y1 = 0.5*(y0 + sc*x/y0)  (one Newton step)
        y0 = stats.tile([128, 1], F32, tag="nw_y0")
        nc.scalar.activation(y0[:sz], x[:sz], AF.Sqrt, scale=sc)
        yr = stats.tile([128, 1], F32, tag="nw_yr")
        nc.vector.tensor_scalar_add(y0[:sz], y0[:sz], 1e-30)
        nc.vector.reciprocal(yr[:sz], y0[:sz])
        nc.vector.tensor_mul(yr[:sz], yr[:sz], x[:sz])
        if sc != 1.0:
            nc.vector.tensor_scalar_mul(yr[:sz], yr[:sz], sc)
        nc.vector.tensor_add(out[:sz], y0[:sz], yr[:sz])
        nc.vector.tensor_scalar_mul(out[:sz], out[:sz], 0.5)

    rr = stats.tile([128, 1], F32, tag="rr")
    _sqrt_newton(rr, r2, 1.0)
    srt = stats.tile([128, 1], F32, tag="srt")
    sia = stats.tile([128, 1], F32, tag="sia")
    nc.vector.tensor_add(tA[:sz], rr[:sz], vre[:sz])
    _sqrt_newton(srt, tA, 0.5)
    nc.vector.tensor_sub(tA[:sz], rr[:sz], vre[:sz])
    _sqrt_newton(sia, tA, 0.5)
    sgn = stats.tile([128, 1], F32, tag="sgn")
    nc.scalar.activation(sgn[:sz], vim[:sz], AF.Sign)
    nc.vector.tensor_mul(sia[:sz], sia[:sz], sgn[:sz])
    rin = stats.tile([128, 1], F32, tag="rin")
    nc.vector.reciprocal(rin[:sz], rr[:sz])
    wr = stats.tile([128, 1], F32, tag="wr")
    wn = stats.tile([128, 1], F32, tag="wn")  # = -w_im
    nc.vector.tensor_mul(wr[:sz], srt[:sz], rin[:sz])
    nc.vector.tensor_mul(wn[:sz], sia[:sz], rin[:sz])
    aT = work.tile([128, C], F32, tag="ln_a")
    bT = work.tile([128, C], F32, tag="ln_b")
    xnr = work.tile([128, C], F32, tag="ln_xnr")
    xni = work.tile([128, C], F32, tag="ln_xni")
    nc.vector.tensor_scalar(aT[:sz], re_sb[:sz], mr[:sz], wr[:sz],
                            ALU.subtract, ALU.mult)
    nc.vector.tensor_scalar(bT[:sz], im_sb[:sz], mi[:sz], wn[:sz],
                            ALU.subtract, ALU.mult)
    nc.vector.tensor_add(xnr[:sz], aT[:sz], bT[:sz])
    nc.vector.tensor_scalar(aT[:sz], re_sb[:sz], mr[:sz], wn[:sz],
                            ALU.subtract, ALU.mult)
    nc.vector.tensor_scalar(bT[:sz], im_sb[:sz], mi[:sz], wr[:sz],
                            ALU.subtract, ALU.mult)
    nc.vector.tensor_sub(xni[:sz], bT[:sz], aT[:sz])
    return xnr, xni


def _body(nc, tc, xT_d, xP_d, wc_d, srb_d, ones_d, wq_d, wk_d, wv_d,
          wpab_d, bkv_d, outT_d, dbg=None):
    ctx = contextlib.ExitStack()
    consts = ctx.enter_context(tc.tile_pool(name="consts", bufs=1))
    big = ctx.enter_context(tc.tile_pool(name="big", bufs=1))
    xqp = ctx.enter_context(tc.tile_pool(name="xqp", bufs=4))
    qd = ctx.enter_context(tc.tile_pool(name="qd", bufs=4))
    qcp = ctx.enter_context(tc.tile_pool(name="qcp", bufs=8))

    # ---- constants ----
    ident = consts.tile([128, 128], F32, tag="ident")
    make_identity(nc, ident)
    ones_col = consts.tile([128, 1], BF16, tag="ones_col")
    nc.vector.memset(ones_col, 1.0)
    ones_h0 = consts.tile([128, 1], BF16, tag="ones_h0")
    nc.vector.memset(ones_h0, 0.0)
    nc.vector.memset(ones_h0[0:64], 1.0)
    ones_h1 = consts.tile([128, 1], BF16, tag="ones_h1")
    nc.vector.memset(ones_h1, 0.0)
    nc.vector.memset(ones_h1[64:128], 1.0)
    ones128 = consts.tile([128, 128], F32, tag="ones128")
    nc.vector.memset(ones128, 1.0)
    ones_row = consts.tile([1, 512], F32R, tag="ones_row")
    nc.sync.dma_start(ones_row[:], ones_d[:])
    nbias = consts.tile([128, 1], F32, tag="nbias")
    nc.vector.memset(nbias, -50.0)

    srb_re = consts.tile([1, C], F32R, tag="srb_re")
    srb_im = consts.tile([1, C], F32R, tag="srb_im")
    nc.sync.dma_start(srb_re[:], srb_d[0:1, :])
    nc.sync.dma_start(srb_im[:], srb_d[1:2, :])
    bk_re = consts.tile([1, 256], F32R, tag="bk_re")
    bk_im = consts.tile([1, 256], F32R, tag="bk_im")
    bv_re = consts.tile([1, 256], F32R, tag="bv_re")
    bv_im = consts.tile([1, 256], F32R, tag="bv_im")
    nc.sync.dma_start(bk_re[:], bkv_d[0:1, 0, :])
    nc.sync.dma_start(bv_re[:], bkv_d[0:1, 1, :])
    nc.sync.dma_start(bk_im[:], bkv_d[1:2, 0, :])
    nc.sync.dma_start(bv_im[:], bkv_d[1:2, 1, :])

    # ---- persistent SBUF ----
    xnTr = big.tile([128, 4, NK], F32R, tag="xnTr")
    xnTi = big.tile([128, 4, NK], F32R, tag="xnTi")
    # kA_h = [kr_h ; -ki_h], kB_h = [ki_h ; kr_h]  (partition dim 64+64)
    kA = big.tile([128, 4, 704], F32R, tag="kA")
    kB = big.tile([128, 4, 704], F32R, tag="kB")
    nc.vector.memset(kA.bitcast(F32)[:, :, 512:576], 0.0)
    nc.vector.memset(kA.bitcast(F32)[:, :, 640:704], 0.0)
    nc.vector.memset(kB.bitcast(F32)[:, :, 512:576], 0.0)
    nc.vector.memset(kB.bitcast(F32)[:, :, 640:704], 0.0)
    vpk = big.tile([128, 5, 4, 128], BF16, tag="vpk")
    wqs = big.tile([128, 3, 4, 256], F32R, tag="wqs")
    nc.sync.dma_start(wqs[:], wq_d.rearrange("s (j p) n -> p s j n", p=128))
    wps = big.tile([128, 2, 4, C], F16, tag="wps")
    nc.sync.dma_start(wps[:], wpab_d.rearrange("a h p n -> p a h n"))

    xT_v = [xT_d[pl].rearrange("(j p) t -> p j t", p=128) for pl in (0, 1)]

    # =====================================================================
    # Phase B: conv (f32r) + LayerNorm + transposes + k/v projections
    # =====================================================================
    bctx = contextlib.ExitStack()
    xs = bctx.enter_context(tc.tile_pool(name="xs", bufs=4))
    wcp = bctx.enter_context(tc.tile_pool(name="wcp", bufs=2))
    ws = bctx.enter_context(tc.tile_pool(name="ws", bufs=2))
    work = bctx.enter_context(tc.tile_pool(name="work", bufs=2))
    stats = bctx.enter_context(tc.tile_pool(name="stats", bufs=2))
    psum = bctx.enter_context(tc.tile_pool(name="psumB", bufs=7, space="PSUM"))

    def emit_qp(q0, nq, pspool):
        """q-projection for one chunk; returns qc[h] = [qr_h;qi_h] tiles."""
        prs = []
        for half in range(2):
            prs.append((pspool.tile([128, 512], F32, tag="bank", name=f"qpr{half}"),
                        pspool.tile([128, 512], F32, tag="bank", name=f"qpi{half}")))
        for cj in range(4):
            xq_r = xqp.tile([128, 512], F32R, tag="xq_r")
            xq_i = xqp.tile([128, 512], F32R, tag="xq_i")
            nc.gpsimd.dma_start(xq_r[:, :nq], xT_v[0][:, cj, q0:q0 + nq])
            nc.sync.dma_start(xq_i[:, :nq], xT_v[1][:, cj, q0:q0 + nq])
            st = cj == 0
            sp = cj == 3
            for half in range(2):
                hs = slice(128 * half, 128 * (half + 1))
                pr, pi = prs[half]
                nc.tensor.matmul(pr[:, :nq], wqs[:, 0, cj, hs], _r(xq_r[:, :nq]),
                                 start=st, stop=False)
                nc.tensor.matmul(pr[:, :nq], wqs[:, 2, cj, hs], _r(xq_i[:, :nq]),
                                 start=False, stop=sp)
                nc.tensor.matmul(pi[:, :nq], wqs[:, 1, cj, hs], _r(xq_r[:, :nq]),
                                 start=st, stop=False)
                nc.tensor.matmul(pi[:, :nq], wqs[:, 0, cj, hs], _r(xq_i[:, :nq]),
                                 start=False, stop=sp)
        qcs = []
        for half in range(2):
            pr, pi = prs[half]
            qr_sb = qd.tile([128, 512], F32R, tag="qd", name=f"qr{half}")
            qi_sb = qd.tile([128, 512], F32R, tag="qd", name=f"qi{half}")
            nc.scalar.copy(qr_sb[:, :nq], pr[:, :nq])
            nc.scalar.copy(qi_sb[:, :nq], pi[:, :nq])
            for j in range(2):
                h = 2 * half + j
                qc = qcp.tile([128, 512], F32R, tag="qc", name=f"qc{h}")
                rs = slice(64 * j, 64 * (j + 1))
                nc.sync.dma_start(qc[0:64, :nq], qr_sb[rs, :nq])
                nc.gpsimd.dma_start(qc[64:128, :nq], qi_sb[rs, :nq])
                if DEBUG and q0 == 0:
                    nc.sync.dma_start(dbg["qc"][h, :, :nq], qc[:, :nq].bitcast(F32))
                qcs.append(qc)
        return qcs


    qcs_early = {Q_CHUNKS[0][0]: emit_qp(*Q_CHUNKS[0], psum)}

    for mg in M_GROUPS:
        tg0 = mg[0][0] * HR
        tgs = sum(nh for _, nh in mg) * HR
        cps = []
        for (hr0, nh) in mg:
            cre = psum.tile([128, C], F32, tag="bank")
            cim = psum.tile([128, C], F32, tag="bank")
            cps.append((cre, cim, hr0 * HR - tg0, hr0 * HR, nh * HR))

        first = [[True, True] for _ in mg]
        for kk in range(16):
            xp_r = xs.tile([128, 3 * 120], F32R, tag="xp_r")
            xp_i = xs.tile([128, 3 * 120], F32R, tag="xp_i")
            nc.gpsimd.dma_start(xp_r[:, :tgs], xP_d[0, 128 * kk:128 * (kk + 1),
                                                    tg0:tg0 + tgs])
            nc.scalar.dma_start(xp_i[:, :tgs], xP_d[1, 128 * kk:128 * (kk + 1),
                                                  tg0:tg0 + tgs])
            w_re = wcp.tile([128, C], F32R, tag="wc_re")
            w_im = wcp.tile([128, C], F32R, tag="wc_im")
            w_in = wcp.tile([128, C], F32R, tag="wc_in")
            nc.scalar.dma_start(w_re[:], wc_d[0, 128 * kk:128 * (kk + 1), :])
            nc.sync.dma_start(w_im[:], wc_d[1, 128 * kk:128 * (kk + 1), :])
            nc.gpsimd.dma_start(w_in[:], wc_d[2, 128 * kk:128 * (kk + 1), :])
            for mi_, (cre, cim, off, t0, sz) in enumerate(cps):
                pat_r = xp_r[:, off:off + sz]
                pat_i = xp_i[:, off:off + sz]
                nc.tensor.matmul(cre[:sz, :], _r(pat_r), _r(w_re[:]),
                                 start=first[mi_][0], stop=False)
                nc.tensor.matmul(cim[:sz, :], _r(pat_r), _r(w_im[:]),
                                 start=first[mi_][1], stop=False)
                first[mi_] = [False, False]
                nc.tensor.matmul(cre[:sz, :], _r(pat_i), _r(w_in[:]),
                                 start=False, stop=False)
                nc.tensor.matmul(cim[:sz, :], _r(pat_i), _r(w_re[:]),
                                 start=False, stop=False)
        for (cre, cim, off, t0, sz) in cps:
            nc.tensor.matmul(cre[:sz, :], _r(ones_row[:, :sz]), _r(srb_re[:]),
                             start=False, stop=True)
            nc.tensor.matmul(cim[:sz, :], _r(ones_row[:, :sz]), _r(srb_im[:]),
                             start=False, stop=True)

        if mg[0][0] != 0:
            # q-projection fills the PE while group-1 LayerNorm runs
            qcs_early[Q_CHUNKS[1][0]] = emit_qp(*Q_CHUNKS[1], psum)

        # ---- LayerNorm + transpose into xnT ----
        for (cre, cim, off, t0, sz) in cps:
            xnr, xni = _ln_chunk(nc, work, stats, cre, cim, sz)
            for cj in range(4):
                for src, dst in ((xnr, xnTr), (xni, xnTi)):
                    pt = psum.tile([128, 128], F32, tag="bank")
                    nc.tensor.transpose(pt[:, :sz],
                                        src[:sz, 128 * cj:128 * (cj + 1)],
                                        ident[:sz, :sz])
                    nc.vector.tensor_copy(dst[:, cj, t0:t0 + sz], pt[:, :sz])

    # ---- k^T projection -> kA/kB (packed K=128 layout) ----
    # kn holds the negated / positive ki pieces that need partition shifts.
    kn = big.tile([128, 2, 704], F32R, tag="kn")
    nc.vector.memset(kn.bitcast(F32)[:, :, 512:576], 0.0)
    nc.vector.memset(kn.bitcast(F32)[:, :, 640:704], 0.0)
    for half in range(2):
        hs = slice(128 * half, 128 * (half + 1))
        p512r = psum.tile([128, 512], F32, tag="bank")
        p512i = psum.tile([128, 512], F32, tag="bank")
        p64r = psum.tile([128, 512], F32, tag="bank")
        p64i = psum.tile([128, 512], F32, tag="bank")
        for cj in range(4):
            wk_r = ws.tile([128, 256], F32R, tag="w_r")
            wk_i = ws.tile([128, 256], F32R, tag="w_i")
            wk_n = ws.tile([128, 256], F32R, tag="w_n")
            nc.scalar.dma_start(wk_r[:], wk_d[0, 128 * cj:128 * (cj + 1), :])
            nc.gpsimd.dma_start(wk_i[:], wk_d[1, 128 * cj:128 * (cj + 1), :])
            nc.sync.dma_start(wk_n[:], wk_d[2, 128 * cj:128 * (cj + 1), :])
            st = cj == 0
            for (pr, pi, n0, nn) in ((p512r, p512i, 0, 512), (p64r, p64i, 512, 64)):
                nc.tensor.matmul(pr[:, :nn], _r(wk_r[:, hs]),
                                 _r(xnTr[:, cj, n0:n0 + nn]), start=st, stop=False)
                nc.tensor.matmul(pr[:, :nn], _r(wk_n[:, hs]),
                                 _r(xnTi[:, cj, n0:n0 + nn]), start=False, stop=False)
                nc.tensor.matmul(pi[:, :nn], _r(wk_i[:, hs]),
                                 _r(xnTr[:, cj, n0:n0 + nn]), start=st, stop=False)
                nc.tensor.matmul(pi[:, :nn], _r(wk_r[:, hs]),
                                 _r(xnTi[:, cj, n0:n0 + nn]), start=False, stop=False)
        h0, h1 = 2 * half, 2 * half + 1
        for (pr, pi, n0, nn) in ((p512r, p512i, 0, 512), (p64r, p64i, 576, 64)):
            nc.tensor.matmul(pr[:, :nn], _r(bk_re[:, hs]), _r(ones_row[:, :nn]),
                             start=False, stop=True)
            nc.tensor.matmul(pi[:, :nn], _r(bk_im[:, hs]), _r(ones_row[:, :nn]),
                             start=False, stop=True)
            ns = slice(n0, n0 + nn)
            # aligned drains (no partition shift):
            nc.vector.tensor_copy(kA[0:64, h0, ns], pr[0:64, :nn])       # kr_h0
            nc.vector.tensor_copy(kB[64:128, h1, ns], pr[64:128, :nn])   # kr_h1
            nc.vector.tensor_copy(kB[0:64, h0, ns], pi[0:64, :nn])       # ki_h0
            nc.vector.tensor_scalar_mul(kA[64:128, h1, ns], pi[64:128, :nn],
                                        -1.0)                            # -ki_h1
            nc.vector.tensor_scalar_mul(kn[0:64, half, ns], pi[0:64, :nn],
                                        -1.0)                            # -ki_h0
            nc.vector.tensor_copy(kn[64:128, half, ns], pi[64:128, :nn])  # ki_h1
        # partition-shifting copies via SBUF->SBUF DMA
        nc.sync.dma_start(kA[64:128, h0, :], kn[0:64, half, :])     # -ki_h0
        nc.sync.dma_start(kB[0:64, h1, :], kn[64:128, half, :])     # ki_h1
        nc.gpsimd.dma_start(kA[0:64, h1, :], kB[64:128, h1, :])     # kr_h1
        nc.gpsimd.dma_start(kB[64:128, h0, :], kA[0:64, h0, :])     # kr_h0

    # ---- v projection (f32r) -> vpk f16 ----
    for kcg in ((0, 1, 2), (3, 4)):
        pps = {}
        for kc in kcg:
            pps[kc] = (psum.tile([128, 512], F32, tag="bank", name=f"vpr{kc}"),
                       psum.tile([128, 512], F32, tag="bank", name=f"vpi{kc}"))
        for cj in range(4):
            wv_r = ws.tile([128, 256], F32R, tag="w_r")
            wv_i = ws.tile([128, 256], F32R, tag="w_i")
            wv_n = ws.tile([128, 256], F32R, tag="w_n")
            nc.scalar.dma_start(wv_r[:], wv_d[0, 128 * cj:128 * (cj + 1), :])
            nc.gpsimd.dma_start(wv_i[:], wv_d[1, 128 * cj:128 * (cj + 1), :])
            nc.sync.dma_start(wv_n[:], wv_d[2, 128 * cj:128 * (cj + 1), :])
            st = cj == 0
            for kc in kcg:
                k0, szk = K_CHUNKS[kc]
                pr, pi = pps[kc]
                nc.tensor.matmul(pr[:szk, :256], _r(xnTr[:, cj, k0:k0 + szk]),
                                 _r(wv_r[:]), start=st, stop=False)
                nc.tensor.matmul(pr[:szk, :256], _r(xnTi[:, cj, k0:k0 + szk]),
                                 _r(wv_n[:]), start=False, stop=False)
                nc.tensor.matmul(pi[:szk, :256], _r(xnTr[:, cj, k0:k0 + szk]),
                                 _r(wv_i[:]), start=st, stop=False)
                nc.tensor.matmul(pi[:szk, :256], _r(xnTi[:, cj, k0:k0 + szk]),
                                 _r(wv_r[:]), start=False, stop=False)
        for kc in kcg:
            k0, szk = K_CHUNKS[kc]
            pr, pi = pps[kc]
            nc.tensor.matmul(pr[:szk, :256], _r(ones_row[:, :szk]), _r(bv_re[:]),
                             start=False, stop=True)
            nc.tensor.matmul(pi[:szk, :256], _r(ones_row[:, :szk]), _r(bv_im[:]),
                             start=False, stop=True)
            vr_v = pr[:szk, :256].rearrange("p (h d) -> p h d", h=4)
            vi_v = pi[:szk, :256].rearrange("p (h d) -> p h d", h=4)
            nc.vector.tensor_copy(vpk[:szk, kc, :, 0:64], vr_v)
            nc.vector.tensor_copy(vpk[:szk, kc, :, 64:128], vi_v)
    # kc4 (szk=64) packs head pairs along partitions in phase C: odd heads
    # read their v from rows 64:128
    nc.sync.dma_start(vpk[64:128, 4, 1, :], vpk[0:64, 4, 1, :])
    nc.gpsimd.dma_start(vpk[64:128, 4, 3, :], vpk[0:64, 4, 3, :])

    if DEBUG:
        for cj in range(4):
            nc.sync.dma_start(dbg["xnT"][0, 128 * cj:128 * (cj + 1), :],
                              xnTr[:, cj, :].bitcast(F32))
            nc.sync.dma_start(dbg["xnT"][1, 128 * cj:128 * (cj + 1), :],
                              xnTi[:, cj, :].bitcast(F32))
        nc.sync.dma_start(dbg["kA"][:], kA.bitcast(F32))
        nc.sync.dma_start(dbg["kB"][:], kB.bitcast(F32))

    bctx.close()

    # =====================================================================
    # Phase C: per q-chunk: q-proj -> scores -> softmax -> attn@v -> proj
    # =====================================================================
    cctx = contextlib.ExitStack()
    sm = cctx.enter_context(tc.tile_pool(name="sm", bufs=2))
    psC = cctx.enter_context(tc.tile_pool(name="psC", bufs=4, space="PSUM"))
    psOp = cctx.enter_context(tc.tile_pool(name="psOp", bufs=2, space="PSUM"))
    psDn = cctx.enter_context(tc.tile_pool(name="psDn", bufs=2, space="PSUM"))

    def emit_front(q0, nq, hp, qcs):
        """Scores + softmax numerators for head-pair hp; returns ebufs.

        kc 0-3: [szk, 2, nq] tiles (head i in dim 1).  kc 4 (szk=64):
        both heads packed in the partition dim (i0 rows 0:64, i1 64:128)
        so every elementwise op runs one [128, nq] pass instead of two.
        """
        sts = []
        for kc in range(4):
            k0, szk = K_CHUNKS[kc]
            s_t = sm.tile([128, 2, 512], F32, tag="s_t", bufs=4,
                          name=f"s{kc}")
            for i in range(2):
                h = 2 * hp + i
                qc = qcs[h]
                sre = psC.tile([128, 512], F32, tag="bank", name="sre")
                sim = psC.tile([128, 512], F32, tag="bank", name="sim")
                nc.tensor.matmul(sre[:szk, :nq], kA[:, h, k0:k0 + szk],
                                 qc[:, :nq], start=True, stop=True)
                nc.tensor.matmul(sim[:szk, :nq], kB[:, h, k0:k0 + szk],
                                 qc[:, :nq], start=True, stop=True)
                # s = sre^2 + sim^2 in f32
                nc.scalar.activation(s_t[:szk, i, :nq], sre[:szk, :nq],
                                     AF.Square)
                tmp = sm.tile([128, 512], F32, tag="tmp", bufs=2)
                if SQMODE == 0:
                    tmc = sm.tile([128, 512], F32, tag="tmc", bufs=2)
                    nc.vector.tensor_copy(tmc[:szk, :nq], sim[:szk, :nq])
                    nc.vector.tensor_mul(tmp[:szk, :nq], tmc[:szk, :nq],
                                         tmc[:szk, :nq])
                    nc.gpsimd.tensor_add(s_t[:szk, i, :nq], s_t[:szk, i, :nq],
                                         tmp[:szk, :nq])
                else:
                    nc.scalar.activation(tmp[:szk, :nq], sim[:szk, :nq],
                                         AF.Square)
                    nc.vector.tensor_add(s_t[:szk, i, :nq], s_t[:szk, i, :nq],
                                         tmp[:szk, :nq])
            if DEBUG and q0 == 0 and hp == 0 and kc < 2:
                nc.sync.dma_start(dbg["s"][kc, :szk, :, :nq], s_t[:szk, :, :nq])
            sts.append(s_t)
        # kc 4: head-pair packed [128, nq]
        s4 = sm.tile([128, 512], F32, tag="s4", bufs=3)
        sre4 = psC.tile([128, 512], F32, tag="bank", name="sre4")
        sim4 = psC.tile([128, 512], F32, tag="bank", name="sim4")
        for i in range(2):
            h = 2 * hp + i
            # i=0: cols [576:704) -> kc4 data lands in out rows 0:64;
            # i=1: cols [512:640) -> rows 64:128; zero cols pad the rest.
            c0 = 576 - 64 * i
            nc.tensor.matmul(sre4[:, :nq], kA[:, h, c0:c0 + 128],
                             qcs[h][:, :nq], start=i == 0, stop=i == 1)
            nc.tensor.matmul(sim4[:, :nq], kB[:, h, c0:c0 + 128],
                             qcs[h][:, :nq], start=i == 0, stop=i == 1)
        nc.scalar.activation(s4[:, :nq], sre4[:, :nq], AF.Square)
        tmp4 = sm.tile([128, 512], F32, tag="tmp", bufs=2)
        if SQMODE == 0:
            tmc4 = sm.tile([128, 512], F32, tag="tmc", bufs=2)
            nc.vector.tensor_copy(tmc4[:, :nq], sim4[:, :nq])
            nc.vector.tensor_mul(tmp4[:, :nq], tmc4[:, :nq], tmc4[:, :nq])
            nc.gpsimd.tensor_add(s4[:, :nq], s4[:, :nq], tmp4[:, :nq])
        else:
            nc.scalar.activation(tmp4[:, :nq], sim4[:, :nq], AF.Square)
            nc.vector.tensor_add(s4[:, :nq], s4[:, :nq], tmp4[:, :nq])
        # batched LUT runs: |a| = sqrt(s) (or exp(0.5 ln s)), then
        # ebuf = exp(|a| - 50) in bf16.  Softmax is shift-invariant so the
        # constant -50 cancels; it just keeps exp sums in range.
        if SQRTMODE == 0:
            for kc in range(4):
                szk = K_CHUNKS[kc][1]
                nc.scalar.activation(sts[kc][:szk, :, :nq],
                                     sts[kc][:szk, :, :nq], AF.Ln)
            nc.scalar.activation(s4[:, :nq], s4[:, :nq], AF.Ln)
            for kc in range(4):
                szk = K_CHUNKS[kc][1]
                nc.scalar.activation(sts[kc][:szk, :, :nq],
                                     sts[kc][:szk, :, :nq], AF.Exp, scale=0.5)
            nc.scalar.activation(s4[:, :nq], s4[:, :nq], AF.Exp, scale=0.5)
        else:
            for kc in range(4):
                szk = K_CHUNKS[kc][1]
                nc.scalar.activation(sts[kc][:szk, :, :nq],
                                     sts[kc][:szk, :, :nq], AF.Sqrt)
            nc.scalar.activation(s4[:, :nq], s4[:, :nq], AF.Sqrt)
        ebufs = []
        for kc in range(4):
            szk = K_CHUNKS[kc][1]
            ebuf = sm.tile([128, 2, 512], BF16, tag="ebuf", bufs=8,
                           name=f"eb{kc}")
            nc.scalar.activation(ebuf[:szk, :, :nq], sts[kc][:szk, :, :nq],
                                 AF.Exp, bias=nbias[:szk])
            ebufs.append(ebuf)
        eb4 = sm.tile([128, 512], BF16, tag="eb4", bufs=3)
        nc.scalar.activation(eb4[:, :nq], s4[:, :nq], AF.Exp, bias=nbias[:])
        ebufs.append(eb4)
        return ebufs

    def emit_back(q0, nq, hp, ebufs):
        """attn@v + denominators + normalize for head-pair hp."""
        ops = [psOp.tile([128, 512], F32, tag="op", name=f"op{i}")
               for i in range(2)]
        dnp = psDn.tile([128, 512], F32, tag="dn", name="dnp")
        for kc in range(4):
            k0, szk = K_CHUNKS[kc]
            ebuf = ebufs[kc]
            for i in range(2):
                h = 2 * hp + i
                nc.tensor.matmul(ops[i][:, :nq], vpk[:szk, kc, h, :],
                                 ebuf[:szk, i, :nq], start=kc == 0, stop=False)
                nc.tensor.matmul(dnp[32 * i:32 * i + 1, :nq], ones_col[:szk, :],
                                 ebuf[:szk, i, :nq], start=kc == 0, stop=False,
                                 tile_position=(0, 32 * i))
        eb4 = ebufs[4]
        for i in range(2):
            h = 2 * hp + i
            rs = slice(64 * i, 64 * (i + 1))
            mask = ones_h0 if i == 0 else ones_h1
            nc.tensor.matmul(ops[i][:, :nq], vpk[rs, 4, h, :],
                             eb4[rs, :nq], start=False, stop=True)
            nc.tensor.matmul(dnp[32 * i:32 * i + 1, :nq], mask[:, :],
                             eb4[:, :nq], start=False, stop=True,
                             tile_position=(0, 32 * i))
        dnt = sm.tile([1, 1024], F32, tag="dnt", bufs=2)
        dns = sm.tile([1, 1024], F32, tag="dns", bufs=2)
        dnr = sm.tile([1, 1024], F32R, tag="dnr", bufs=2)
        for i in range(2):
            c = slice(512 * i, 512 * i + nq)
            # ACT partition-shifts row 32i of PSUM into partition 0;
            # reciprocal_approx_fast is broken for base_partition != 0 on HW
            nc.scalar.copy(dnt[0:1, c], dnp[32 * i:32 * i + 1, :nq])
            if RECIPMODE == 0:
                nc.vector.reciprocal_approx_fast(dns[0:1, c], dnt[0:1, c])
                nc.vector.tensor_copy(dnr[0:1, c], dns[0:1, c])
            else:
                nc.scalar.activation(dns[0:1, c], dnt[0:1, c], AF.Ln)
                nc.scalar.activation(dnr[0:1, c], dns[0:1, c], AF.Exp,
                                     scale=-1.0)
        oris = []
        for i in range(2):
            osb = sm.tile([128, 512], F32, tag="osb", bufs=2, name=f"osb{i}")
            nc.scalar.copy(osb[:, :nq], ops[i][:, :nq])
            rbp = psC.tile([128, 512], F32, tag="bank", name="rbp")
            nc.tensor.matmul(rbp[:, :nq], _r(ones128[0:1, :]),
                             dnr[0:1, 512 * i:512 * i + nq],
                             start=True, stop=True)
            ori = sm.tile([128, 512], F16, tag="ori", bufs=5, name=f"ori{i}")
            nc.vector.tensor_mul(ori[:, :nq], osb[:, :nq], rbp[:, :nq])
            oris.append(ori)
        return oris

    def emit_proj(q0, nq, oris):
        """Partial output projection for one chunk; oris = [h0..h3]."""
        for cc in range(4):
            cs = slice(128 * cc, 128 * (cc + 1))
            pr = psC.tile([128, 512], F32, tag="bank", name="pjr")
            pi = psC.tile([128, 512], F32, tag="bank", name="pji")
            for h in range(4):
                st = h == 0
                sp = h == 3
                nc.tensor.matmul(pr[:, :nq], wps[:, 0, h, cs], oris[h][:, :nq],
                                 start=st, stop=sp)
                nc.tensor.matmul(pi[:, :nq], wps[:, 1, h, cs], oris[h][:, :nq],
                                 start=st, stop=sp)
            o1 = sm.tile([128, 512], F16, tag="o1", bufs=2)
            o2 = sm.tile([128, 512], F16, tag="o2", bufs=2)
            nc.vector.tensor_copy(o1[:, :nq], pr[:, :nq])
            nc.vector.tensor_copy(o2[:, :nq], pi[:, :nq])
            nc.gpsimd.dma_start(outT_d[0, cs, q0:q0 + nq], o1[:, :nq])
            nc.sync.dma_start(outT_d[1, cs, q0:q0 + nq], o2[:, :nq])

    # software pipeline, two half-steps deep: back(H[i-2]) is emitted after
    # front(H[i]) so its ebufs have had two full half-steps of ACT/DVE time;
    # the PE meanwhile streams sc(H[i]) + av(H[i-2]) + qp/proj with no stall.
    halves = [(q0, nq, hp) for (q0, nq) in Q_CHUNKS for hp in range(2)]
    qcs_by_q0 = dict(qcs_early)
    fronts = []
    ori_by_q0 = {}

    def _drain(idx):
        bq0, bnq, bhp, bebufs = fronts[idx]
        oris = emit_back(bq0, bnq, bhp, bebufs)
        ori_by_q0.setdefault(bq0, []).extend(oris)
        if bhp == 1:
            emit_proj(bq0, bnq, ori_by_q0.pop(bq0))

    for i, (q0, nq, hp) in enumerate(halves):
        if i >= 2:
            _drain(i - 2)
        ebufs = emit_front(q0, nq, hp, qcs_by_q0[q0])
        fronts.append((q0, nq, hp, ebufs))
        qi = i // 2
        if hp == 1 and qi + 2 < len(Q_CHUNKS):
            nq0, nnq = Q_CHUNKS[qi + 2]
            qcs_by_q0[nq0] = emit_qp(nq0, nnq, psC)
    _drain(len(halves) - 2)
    _drain(len(halves) - 1)

    cctx.close()
    ctx.close()


# =========================================================================
# Host side
# =========================================================================

def _f32(x):
    return np.ascontiguousarray(x, dtype=np.float32)


def _f16(x):
    return np.ascontiguousarray(np.asarray(x, dtype=np.float16))


def host_prep(x_re, x_im, Wq, Wkv, Wproj, bproj, sr_w, sr_b, gain, bias):
    x_re = np.asarray(x_re)
    x_im = np.asarray(x_im)
    Wq = np.asarray(Wq)
    Wkv = np.asarray(Wkv)
    Wproj = np.asarray(Wproj)
    sr_w = np.asarray(sr_w)
    sr_b = np.asarray(sr_b)
    gain = np.asarray(gain)
    bias = np.asarray(bias)

    Wkv_eff = gain[:, None] * Wkv
    bkv_full = bias @ Wkv
    Wc = sr_w.transpose(2, 3, 1, 0).reshape(4 * C, C)

    def planes3f(w):
        return np.stack([_f32(w.real), _f32(w.imag), _f32(-w.imag)])

    in_maps = []
    for core in range(8):
        b, g = core // 2, core % 2
        cols = slice(256 * g, 256 * (g + 1))
        wk_c = Wkv_eff[:, :C][:, cols] * SCALE
        wv_c = Wkv_eff[:, C:][:, cols]
        bk_c = bkv_full[:C][cols] * SCALE
        bv_c = bkv_full[C:][cols]
        xs_c = np.stack([x_re[b].T, x_im[b].T])  # [2, C, N]
        xsp = xs_c.reshape(2, C, HR, 2, HR, 2)
        xP = np.stack([xsp[:, :, :, p, :, q].reshape(2, C, NK)
                       for p in range(2) for q in range(2)], axis=1)
        # wpA_h = [Wp_re rows of head h ; -Wp_im rows], wpB_h = [im ; re]
        wp_blk = Wproj[256 * g:256 * (g + 1), :]  # [256, C] complex
        wpab = np.zeros((2, 4, 128, C), np.float16)
        for h in range(4):
            rows = wp_blk[64 * h:64 * (h + 1), :]
            wpab[0, h, 0:64] = _f16(rows.real)
            wpab[0, h, 64:128] = _f16(-rows.imag)
            wpab[1, h, 0:64] = _f16(rows.imag)
            wpab[1, h, 64:128] = _f16(rows.real)
        m = {
            "xT": _f32(xs_c),
            "xP": _f32(xP.reshape(2, 4 * C, NK)),
            "wc": planes3f(Wc),
            "srb": np.stack([_f32(sr_b.real), _f32(sr_b.imag)]),
            "ones": np.ones((1, 512), np.float32),
            "wq": planes3f(Wq[:, cols]),
            "wk": planes3f(wk_c),
            "wv": planes3f(wv_c),
            "wpab": wpab,
            "bkv": np.stack([
                np.stack([_f32(bk_c.real), _f32(bv_c.real)]),
                np.stack([_f32(bk_c.imag), _f32(bv_c.imag)]),
            ]),
        }
        in_maps.append(m)
    return in_maps


_NC_CACHE = None


def _get_nc():
    global _NC_CACHE
    if _NC_CACHE is None:
        _NC_CACHE = build_nc()
    return _NC_CACHE


def kernel(x_re, x_im, Wq, Wkv, Wproj, bproj, sr_w, sr_b, gain, bias, H, W):
    from concourse.bass_utils import run_bass_kernel_spmd

    nc = _get_nc()
    in_maps = host_prep(x_re, x_im, Wq, Wkv, Wproj, bproj, sr_w, sr_b, gain, bias)
    res = run_bass_kernel_spmd(nc, in_maps, list(range(8)))
    bproj = np.asarray(bproj)
    out = np.zeros((B, N, C), dtype=np.complex64)
    for b in range(B):
        p0 = res.results[2 * b]["outT"].astype(np.float32)
        p1 = res.results[2 * b + 1]["outT"].astype(np.float32)
        acc = (p0[0] + p1[0]).T + 1j * (p0[1] + p1[1]).T
        out[b] = acc + bproj[None, :]
    return out


# revision 24
# speedup vs baseline: 1.1362x; 1.1362x over previous
"""Trainium2 Bass kernel for complex-valued spatial-reduction attention.

x: [B=4, N=2304, C=512] complex64 (re/im f32 planes), H=W=48, 8 heads,
head_dim 64, sr_ratio 2 -> Nk=576.

Sharding: 8 cores = 4 batches x 2 head-groups (4 heads each). Each core:
sr-conv over full C, complex LayerNorm, q/k/v for its heads,
softmax(|q.k^T|) attention, attn @ v, partial output projection.
Host sums the two partials per batch and adds bproj.

v2 structure: scores use K=128 packing (kA=[kr;-ki], kB=[ki;kr],
qcat=[qr;qi]); q-projection and output projection run inside the
attention loop (q/attn-out never round-trip DRAM); softmax runs
Square(ACT f32) + sim^2/add (DVE f32) + batched Ln/Exp runs;
denominator reciprocal on DVE (no act-table thrash); v/attn/proj in f16.
"""

import os
import contextlib

import numpy as np
import ml_dtypes

import concourse.bass as bass
import concourse.mybir as mybir
import concourse.tile as tile
from concourse import bacc
from concourse.masks import make_identity

BF16 = mybir.dt.bfloat16
F16 = mybir.dt.float16
F32 = mybir.dt.float32
F32R = mybir.dt.float32r
AF = mybir.ActivationFunctionType
ALU = mybir.AluOpType

B, N, C, HEADS, HD, SR = 4, 2304, 512, 8, 64, 2
NK = 576
HR = 24
EPS = 1e-5
SCALE = HD ** -0.5  # folded into Wk host-side

M_GROUPS = [
    [(0, 5), (5, 5), (10, 5)],
    [(15, 5), (20, 4)],
]
K_CHUNKS = [(0, 128), (128, 128), (256, 128), (384, 128), (512, 64)]
Q_CHUNKS = [(0, 512), (512, 512), (1024, 512), (1536, 512), (2048, 256)]

# 0 = ln/exp sqrt (safe), 1 = ACT Sqrt LUT (faster; precision probe)
SQRTMODE = int(os.environ.get("KBUILD_SQRT", "1"))
# 0 = DVE tensor_mul(sim, sim) reading same PSUM AP twice; 1 = ACT Square
SQMODE = int(os.environ.get("KBUILD_SQ", "0"))
DEBUG = bool(int(os.environ.get("KBUILD_DEBUG", "0")))
# 1 = baseline ln/exp reciprocal instead of reciprocal_approx_fast
RECIPMODE = int(os.environ.get("KBUILD_RECIP", "0"))


def _r(ap):
    return ap.bitcast(F32R)


def build_nc():
    nc = bacc.Bacc("TRN2", target_bir_lowering=False, debug=False, num_devices=8)

    xT_d = nc.dram_tensor("xT", [2, C, N], F32R, kind="ExternalInput")
    xP_d = nc.dram_tensor("xP", [2, 4 * C, NK], F32R, kind="ExternalInput")
    wc_d = nc.dram_tensor("wc", [3, 4 * C, C], F32R, kind="ExternalInput")
    srb_d = nc.dram_tensor("srb", [2, C], F32R, kind="ExternalInput")
    ones_d = nc.dram_tensor("ones", [1, 512], F32R, kind="ExternalInput")
    wq_d = nc.dram_tensor("wq", [3, C, 256], F32R, kind="ExternalInput")
    wk_d = nc.dram_tensor("wk", [3, C, 256], F32R, kind="ExternalInput")
    wv_d = nc.dram_tensor("wv", [3, C, 256], F32R, kind="ExternalInput")
    wpab_d = nc.dram_tensor("wpab", [2, 4, 128, C], F16, kind="ExternalInput")
    bkv_d = nc.dram_tensor("bkv", [2, 2, 256], F32R, kind="ExternalInput")
    outT_d = nc.dram_tensor("outT", [2, C, N], F16, kind="ExternalOutput")
    dbg = {}
    if DEBUG:
        dbg["kA"] = nc.dram_tensor("dbg_kA", [128, 4, NK], F32, kind="ExternalOutput")
        dbg["kB"] = nc.dram_tensor("dbg_kB", [128, 4, NK], F32, kind="ExternalOutput")
        dbg["qc"] = nc.dram_tensor("dbg_qc", [4, 128, 512], F32, kind="ExternalOutput")
        dbg["s"] = nc.dram_tensor("dbg_s", [2, 128, 2, 512], F32, kind="ExternalOutput")
        dbg["xnT"] = nc.dram_tensor("dbg_xnT", [2, C, NK], F32, kind="ExternalOutput")

    with tile.TileContext(nc) as tc:
        _body(nc, tc, xT_d, xP_d, wc_d, srb_d, ones_d, wq_d, wk_d, wv_d,
              wpab_d, bkv_d, outT_d, dbg)

    nc.compile()
    return nc


def _ln_chunk(nc, work, stats, cre, cim, sz):
    """Complex LayerNorm for one [sz, C] chunk in PSUM -> (xnr, xni)."""
    inv_c = 1.0 / C
    re_sb = work.tile([128, C], F32, tag="ln_re")
    im_sb = work.tile([128, C], F32, tag="ln_im")
    sum_r = stats.tile([128, 1], F32, tag="sum_r")
    sum_i = stats.tile([128, 1], F32, tag="sum_i")
    nc.vector.tensor_copy(re_sb[:sz], cre[:sz, :])
    nc.vector.tensor_copy(im_sb[:sz], cim[:sz, :])
    nc.vector.tensor_reduce(sum_r[:sz], re_sb[:sz], mybir.AxisListType.X, ALU.add)
    nc.vector.tensor_reduce(sum_i[:sz], im_sb[:sz], mybir.AxisListType.X, ALU.add)
    junk = work.tile([128, C], F32, tag="ln_junk", bufs=1)
    sxx = stats.tile([128, 1], F32, tag="sxx")
    sii = stats.tile([128, 1], F32, tag="sii")
    sxi = stats.tile([128, 1], F32, tag="sxi")
    nc.vector.tensor_mul(junk[:sz], re_sb[:sz], re_sb[:sz])
    nc.vector.tensor_reduce(sxx[:sz], junk[:sz], mybir.AxisListType.X, ALU.add)
    nc.vector.tensor_mul(junk[:sz], im_sb[:sz], im_sb[:sz])
    nc.vector.tensor_reduce(sii[:sz], junk[:sz], mybir.AxisListType.X, ALU.add)
    nc.vector.tensor_mul(junk[:sz], re_sb[:sz], im_sb[:sz])
    nc.vector.tensor_reduce(sxi[:sz], junk[:sz], mybir.AxisListType.X, ALU.add)
    mr = stats.tile([128, 1], F32, tag="mr")
    mi = stats.tile([128, 1], F32, tag="mi")
    nc.vector.tensor_scalar_mul(mr[:sz], sum_r[:sz], inv_c)
    nc.vector.tensor_scalar_mul(mi[:sz], sum_i[:sz], inv_c)
    vre = stats.tile([128, 1], F32, tag="vre")
    vim = stats.tile([128, 1], F32, tag="vim")
    tA = stats.tile([128, 1], F32, tag="tA")
    tB = stats.tile([128, 1], F32, tag="tB")
    nc.vector.tensor_sub(tA[:sz], sxx[:sz], sii[:sz])
    nc.vector.tensor_scalar_mul(tA[:sz], tA[:sz], inv_c)
    nc.vector.tensor_mul(vre[:sz], mr[:sz], mr[:sz])
    nc.vector.tensor_mul(tB[:sz], mi[:sz], mi[:sz])
    nc.vector.tensor_sub(vre[:sz], vre[:sz], tB[:sz])
    nc.vector.tensor_sub(vre[:sz], tA[:sz], vre[:sz])
    nc.vector.tensor_scalar_add(vre[:sz], vre[:sz], EPS)
    nc.vector.tensor_mul(tB[:sz], mr[:sz], mi[:sz])
    nc.vector.tensor_scalar_mul(tB[:sz], tB[:sz], 2.0)
    nc.vector.tensor_scalar_mul(vim[:sz], sxi[:sz], 2.0 * inv_c)
    nc.vector.tensor_sub(vim[:sz], vim[:sz], tB[:sz])
    r2 = stats.tile([128, 1], F32, tag="r2")
    nc.vector.tensor_mul(r2[:sz], vre[:sz], vre[:sz])
    nc.vector.tensor_mul(tB[:sz], vim[:sz], vim[:sz])
    nc.vector.tensor_add(r2[:sz], r2[:sz], tB[:sz])

    def _sqrt_newton(out, x, sc):
        # y0 = LUT sqrt(sc*x); y1 = 0.5*(y0 + sc*x/y0)  (one Newton step)
        y0 = stats.tile([128, 1], F32, tag="nw_y0")
        nc.scalar.activation(y0[:sz], x[:sz], AF.Sqrt, scale=sc)
        yr = stats.tile([128, 1], F32, tag="nw_yr")
        nc.vector.tensor_scalar_add(y0[:sz], y0[:sz], 1e-30)
        nc.vector.reciprocal(yr[:sz], y0[:sz])
        nc.vector.tensor_mul(yr[:sz], yr[:sz], x[:sz])
        if sc != 1.0:
            nc.vector.tensor_scalar_mul(yr[:sz], yr[:sz], sc)
        nc.vector.tensor_add(out[:sz], y0[:sz], yr[:sz])
        nc.vector.tensor_scalar_mul(out[:sz], out[:sz], 0.5)

    rr = stats.tile([128, 1], F32, tag="rr")
    _sqrt_newton(rr, r2, 1.0)
    srt = stats.tile([128, 1], F32, tag="srt")
    sia = stats.tile([128, 1], F32, tag="sia")
    nc.vector.tensor_add(tA[:sz], rr[:sz], vre[:sz])
    _sqrt_newton(srt, tA, 0.5)
    nc.vector.tensor_sub(tA[:sz], rr[:sz], vre[:sz])
    _sqrt_newton(sia, tA, 0.5)
    sgn = stats.tile([128, 1], F32, tag="sgn")
    nc.scalar.activation(sgn[:sz], vim[:sz], AF.Sign)
    nc.vector.tensor_mul(sia[:sz], sia[:sz], sgn[:sz])
    rin = stats.tile([128, 1], F32, tag="rin")
    nc.vector.reciprocal(rin[:sz], rr[:sz])
    wr = stats.tile([128, 1], F32, tag="wr")
    wn = stats.tile([128, 1], F32, tag="wn")  # = -w_im
    nc.vector.tensor_mul(wr[:sz], srt[:sz], rin[:sz])
    nc.vector.tensor_mul(wn[:sz], sia[:sz], rin[:sz])
    aT = work.tile([128, C], F32, tag="ln_a")
    bT = work.tile([128, C], F32, tag="ln_b")
    xnr = work.tile([128, C], F32, tag="ln_xnr")
    xni = work.tile([128, C], F32, tag="ln_xni")
    nc.vector.tensor_scalar(aT[:sz], re_sb[:sz], mr[:sz], wr[:sz],
                            ALU.subtract, ALU.mult)
    nc.vector.tensor_scalar(bT[:sz], im_sb[:sz], mi[:sz], wn[:sz],
                            ALU.subtract, ALU.mult)
    nc.vector.tensor_add(xnr[:sz], aT[:sz], bT[:sz])
    nc.vector.tensor_scalar(aT[:sz], re_sb[:sz], mr[:sz], wn[:sz],
                            ALU.subtract, ALU.mult)
    nc.vector.tensor_scalar(bT[:sz], im_sb[:sz], mi[:sz], wr[:sz],
                            ALU.subtract, ALU.mult)
    nc.vector.tensor_sub(xni[:sz], bT[:sz], aT[:sz])
    return xnr, xni


def _body(nc, tc, xT_d, xP_d, wc_d, srb_d, ones_d, wq_d, wk_d, wv_d,
          wpab_d, bkv_d, outT_d, dbg=None):
    ctx = contextlib.ExitStack()
    consts = ctx.enter_context(tc.tile_pool(name="consts", bufs=1))
    big = ctx.enter_context(tc.tile_pool(name="big", bufs=1))
    xqp = ctx.enter_context(tc.tile_pool(name="xqp", bufs=4))
    qd = ctx.enter_context(tc.tile_pool(name="qd", bufs=4))
    qcp = ctx.enter_context(tc.tile_pool(name="qcp", bufs=8))

    # ---- constants ----
    ident = consts.tile([128, 128], F32, tag="ident")
    make_identity(nc, ident)
    ones_col = consts.tile([128, 1], BF16, tag="ones_col")
    nc.vector.memset(ones_col, 1.0)
    ones_h0 = consts.tile([128, 1], BF16, tag="ones_h0")
    nc.vector.memset(ones_h0, 0.0)
    nc.vector.memset(ones_h0[0:64], 1.0)
    ones_h1 = consts.tile([128, 1], BF16, tag="ones_h1")
    nc.vector.memset(ones_h1, 0.0)
    nc.vector.memset(ones_h1[64:128], 1.0)
    ones128 = consts.tile([128, 128], F32, tag="ones128")
    nc.vector.memset(ones128, 1.0)
    ones_row = consts.tile([1, 512], F32R, tag="ones_row")
    nc.sync.dma_start(ones_row[:], ones_d[:])
    nbias = consts.tile([128, 1], F32, tag="nbias")
    nc.vector.memset(nbias, -50.0)

    srb_re = consts.tile([1, C], F32R, tag="srb_re")
    srb_im = consts.tile([1, C], F32R, tag="srb_im")
    nc.sync.dma_start(srb_re[:], srb_d[0:1, :])
    nc.sync.dma_start(srb_im[:], srb_d[1:2, :])
    bk_re = consts.tile([1, 256], F32R, tag="bk_re")
    bk_im = consts.tile([1, 256], F32R, tag="bk_im")
    bv_re = consts.tile([1, 256], F32R, tag="bv_re")
    bv_im = consts.tile([1, 256], F32R, tag="bv_im")
    nc.sync.dma_start(bk_re[:], bkv_d[0:1, 0, :])
    nc.sync.dma_start(bv_re[:], bkv_d[0:1, 1, :])
    nc.sync.dma_start(bk_im[:], bkv_d[1:2, 0, :])
    nc.sync.dma_start(bv_im[:], bkv_d[1:2, 1, :])

    # ---- persistent SBUF ----
    xnTr = big.tile([128, 4, NK], F32R, tag="xnTr")
    xnTi = big.tile([128, 4, NK], F32R, tag="xnTi")
    # kA_h = [kr_h ; -ki_h], kB_h = [ki_h ; kr_h]  (partition dim 64+64)
    kA = big.tile([128, 4, 704], F32R, tag="kA")
    kB = big.tile([128, 4, 704], F32R, tag="kB")
    nc.vector.memset(kA.bitcast(F32)[:, :, 512:576], 0.0)
    nc.vector.memset(kA.bitcast(F32)[:, :, 640:704], 0.0)
    nc.vector.memset(kB.bitcast(F32)[:, :, 512:576], 0.0)
    nc.vector.memset(kB.bitcast(F32)[:, :, 640:704], 0.0)
    vpk = big.tile([128, 5, 4, 128], BF16, tag="vpk")
    wqs = big.tile([128, 3, 4, 256], F32R, tag="wqs")
    nc.sync.dma_start(wqs[:], wq_d.rearrange("s (j p) n -> p s j n", p=128))
    wps = big.tile([128, 2, 4, C], F16, tag="wps")
    nc.sync.dma_start(wps[:], wpab_d.rearrange("a h p n -> p a h n"))

    xT_v = [xT_d[pl].rearrange("(j p) t -> p j t", p=128) for pl in (0, 1)]

    # =====================================================================
    # Phase B: conv (f32r) + LayerNorm + transposes + k/v projections
    # =====================================================================
    bctx = contextlib.ExitStack()
    xs = bctx.enter_context(tc.tile_pool(name="xs", bufs=4))
    wcp = bctx.enter_context(tc.tile_pool(name="wcp", bufs=2))
    ws = bctx.enter_context(tc.tile_pool(name="ws", bufs=2))
    work = bctx.enter_context(tc.tile_pool(name="work", bufs=2))
    stats = bctx.enter_context(tc.tile_pool(name="stats", bufs=2))
    psum = bctx.enter_context(tc.tile_pool(name="psumB", bufs=7, space="PSUM"))

    def emit_qp(q0, nq, pspool):
        """q-projection for one chunk; returns qc[h] = [qr_h;qi_h] tiles."""
        prs = []
        for half in range(2):
            prs.append((pspool.tile([128, 512], F32, tag="bank", name=f"qpr{half}"),
                        pspool.tile([128, 512], F32, tag="bank", name=f"qpi{half}")))
        for cj in range(4):
            xq_r = xqp.tile([128, 512], F32R, tag="xq_r")
            xq_i = xqp.tile([128, 512], F32R, tag="xq_i")
            nc.gpsimd.dma_start(xq_r[:, :nq], xT_v[0][:, cj, q0:q0 + nq])
            nc.sync.dma_start(xq_i[:, :nq], xT_v[1][:, cj, q0:q0 + nq])
            st = cj == 0
            sp = cj == 3
            for half in range(2):
                hs = slice(128 * half, 128 * (half + 1))
                pr, pi = prs[half]
                nc.tensor.matmul(pr[:, :nq], wqs[:, 0, cj, hs], _r(xq_r[:, :nq]),
                                 start=st, stop=False)
                nc.tensor.matmul(pr[:, :nq], wqs[:, 2, cj, hs], _r(xq_i[:, :nq]),
                                 start=False, stop=sp)
                nc.tensor.matmul(pi[:, :nq], wqs[:, 1, cj, hs], _r(xq_r[:, :nq]),
                                 start=st, stop=False)
                nc.tensor.matmul(pi[:, :nq], wqs[:, 0, cj, hs], _r(xq_i[:, :nq]),
                                 start=False, stop=sp)
        qcs = []
        for half in range(2):
            pr, pi = prs[half]
            qr_sb = qd.tile([128, 512], F32R, tag="qd", name=f"qr{half}")
            qi_sb = qd.tile([128, 512], F32R, tag="qd", name=f"qi{half}")
            nc.scalar.copy(qr_sb[:, :nq], pr[:, :nq])
            nc.scalar.copy(qi_sb[:, :nq], pi[:, :nq])
            for j in range(2):
                h = 2 * half + j
                qc = qcp.tile([128, 512], F32R, tag="qc", name=f"qc{h}")
                rs = slice(64 * j, 64 * (j + 1))
                nc.sync.dma_start(qc[0:64, :nq], qr_sb[rs, :nq])
                nc.gpsimd.dma_start(qc[64:128, :nq], qi_sb[rs, :nq])
                if DEBUG and q0 == 0:
                    nc.sync.dma_start(dbg["qc"][h, :, :nq], qc[:, :nq].bitcast(F32))
                qcs.append(qc)
        return qcs


    qcs_early = {Q_CHUNKS[0][0]: emit_qp(*Q_CHUNKS[0], psum)}

    for mg in M_GROUPS:
        tg0 = mg[0][0] * HR
        tgs = sum(nh for _, nh in mg) * HR
        cps = []
        for (hr0, nh) in mg:
            cre = psum.tile([128, C], F32, tag="bank")
            cim = psum.tile([128, C], F32, tag="bank")
            cps.append((cre, cim, hr0 * HR - tg0, hr0 * HR, nh * HR))

        first = [[True, True] for _ in mg]
        for kk in range(16):
            xp_r = xs.tile([128, 3 * 120], F32R, tag="xp_r")
            xp_i = xs.tile([128, 3 * 120], F32R, tag="xp_i")
            nc.gpsimd.dma_start(xp_r[:, :tgs], xP_d[0, 128 * kk:128 * (kk + 1),
                                                    tg0:tg0 + tgs])
            nc.scalar.dma_start(xp_i[:, :tgs], xP_d[1, 128 * kk:128 * (kk + 1),
                                                  tg0:tg0 + tgs])
            w_re = wcp.tile([128, C], F32R, tag="wc_re")
            w_im = wcp.tile([128, C], F32R, tag="wc_im")
            w_in = wcp.tile([128, C], F32R, tag="wc_in")
            nc.scalar.dma_start(w_re[:], wc_d[0, 128 * kk:128 * (kk + 1), :])
            nc.sync.dma_start(w_im[:], wc_d[1, 128 * kk:128 * (kk + 1), :])
            nc.gpsimd.dma_start(w_in[:], wc_d[2, 128 * kk:128 * (kk + 1), :])
            for mi_, (cre, cim, off, t0, sz) in enumerate(cps):
                pat_r = xp_r[:, off:off + sz]
                pat_i = xp_i[:, off:off + sz]
                nc.tensor.matmul(cre[:sz, :], _r(pat_r), _r(w_re[:]),
                                 start=first[mi_][0], stop=False)
                nc.tensor.matmul(cim[:sz, :], _r(pat_r), _r(w_im[:]),
                                 start=first[mi_][1], stop=False)
                first[mi_] = [False, False]
                nc.tensor.matmul(cre[:sz, :], _r(pat_i), _r(w_in[:]),
                                 start=False, stop=False)
                nc.tensor.matmul(cim[:sz, :], _r(pat_i), _r(w_re[:]),
                                 start=False, stop=False)
        for (cre, cim, off, t0, sz) in cps:
            nc.tensor.matmul(cre[:sz, :], _r(ones_row[:, :sz]), _r(srb_re[:]),
                             start=False, stop=True)
            nc.tensor.matmul(cim[:sz, :], _r(ones_row[:, :sz]), _r(srb_im[:]),
                             start=False, stop=True)

        if mg[0][0] != 0:
            # q-projection fills the PE while group-1 LayerNorm runs
            qcs_early[Q_CHUNKS[1][0]] = emit_qp(*Q_CHUNKS[1], psum)

        # ---- LayerNorm + transpose into xnT ----
        for (cre, cim, off, t0, sz) in cps:
            xnr, xni = _ln_chunk(nc, work, stats, cre, cim, sz)
            for cj in range(4):
                for src, dst in ((xnr, xnTr), (xni, xnTi)):
                    pt = psum.tile([128, 128], F32, tag="bank")
                    nc.tensor.transpose(pt[:, :sz],
                                        src[:sz, 128 * cj:128 * (cj + 1)],
                                        ident[:sz, :sz])
                    nc.vector.tensor_copy(dst[:, cj, t0:t0 + sz], pt[:, :sz])

    # ---- k^T projection -> kA/kB (packed K=128 layout) ----
    # kn holds the negated / positive ki pieces that need partition shifts.
    kn = big.tile([128, 2, 704], F32R, tag="kn")
    nc.vector.memset(kn.bitcast(F32)[:, :, 512:576], 0.0)
    nc.vector.memset(kn.bitcast(F32)[:, :, 640:704], 0.0)
    for half in range(2):
        hs = slice(128 * half, 128 * (half + 1))
        p512r = psum.tile([128, 512], F32, tag="bank")
        p512i = psum.tile([128, 512], F32, tag="bank")
        p64r = psum.tile([128, 512], F32, tag="bank")
        p64i = psum.tile([128, 512], F32, tag="bank")
        for cj in range(4):
            wk_r = ws.tile([128, 256], F32R, tag="w_r")
            wk_i = ws.tile([128, 256], F32R, tag="w_i")
            wk_n = ws.tile([128, 256], F32R, tag="w_n")
            nc.scalar.dma_start(wk_r[:], wk_d[0, 128 * cj:128 * (cj + 1), :])
            nc.gpsimd.dma_start(wk_i[:], wk_d[1, 128 * cj:128 * (cj + 1), :])
            nc.sync.dma_start(wk_n[:], wk_d[2, 128 * cj:128 * (cj + 1), :])
            st = cj == 0
            for (pr, pi, n0, nn) in ((p512r, p512i, 0, 512), (p64r, p64i, 512, 64)):
                nc.tensor.matmul(pr[:, :nn], _r(wk_r[:, hs]),
                                 _r(xnTr[:, cj, n0:n0 + nn]), start=st, stop=False)
                nc.tensor.matmul(pr[:, :nn], _r(wk_n[:, hs]),
                                 _r(xnTi[:, cj, n0:n0 + nn]), start=False, stop=False)
                nc.tensor.matmul(pi[:, :nn], _r(wk_i[:, hs]),
                                 _r(xnTr[:, cj, n0:n0 + nn]), start=st, stop=False)
                nc.tensor.matmul(pi[:, :nn], _r(wk_r[:, hs]),
                                 _r(xnTi[:, cj, n0:n0 + nn]), start=False, stop=False)
        h0, h1 = 2 * half, 2 * half + 1
        for (pr, pi, n0, nn) in ((p512r, p512i, 0, 512), (p64r, p64i, 576, 64)):
            nc.tensor.matmul(pr[:, :nn], _r(bk_re[:, hs]), _r(ones_row[:, :nn]),
                             start=False, stop=True)
            nc.tensor.matmul(pi[:, :nn], _r(bk_im[:, hs]), _r(ones_row[:, :nn]),
                             start=False, stop=True)
            ns = slice(n0, n0 + nn)
            # aligned drains (no partition shift):
            nc.vector.tensor_copy(kA[0:64, h0, ns], pr[0:64, :nn])       # kr_h0
            nc.vector.tensor_copy(kB[64:128, h1, ns], pr[64:128, :nn])   # kr_h1
            nc.vector.tensor_copy(kB[0:64, h0, ns], pi[0:64, :nn])       # ki_h0
            nc.vector.tensor_scalar_mul(kA[64:128, h1, ns], pi[64:128, :nn],
                                        -1.0)                            # -ki_h1
            nc.vector.tensor_scalar_mul(kn[0:64, half, ns], pi[0:64, :nn],
                                        -1.0)                            # -ki_h0
            nc.vector.tensor_copy(kn[64:128, half, ns], pi[64:128, :nn])  # ki_h1
        # partition-shifting copies via SBUF->SBUF DMA
        nc.sync.dma_start(kA[64:128, h0, :], kn[0:64, half, :])     # -ki_h0
        nc.sync.dma_start(kB[0:64, h1, :], kn[64:128, half, :])     # ki_h1
        nc.gpsimd.dma_start(kA[0:64, h1, :], kB[64:128, h1, :])     # kr_h1
        nc.gpsimd.dma_start(kB[64:128, h0, :], kA[0:64, h0, :])     # kr_h0

    # ---- v projection (f32r) -> vpk f16 ----
    for kcg in ((0, 1, 2), (3, 4)):
        pps = {}
        for kc in kcg:
            pps[kc] = (psum.tile([128, 512], F32, tag="bank", name=f"vpr{kc}"),
                       psum.tile([128, 512], F32, tag="bank", name=f"vpi{kc}"))
        for cj in range(4):
            wv_r = ws.tile([128, 256], F32R, tag="w_r")
            wv_i = ws.tile([128, 256], F32R, tag="w_i")
            wv_n = ws.tile([128, 256], F32R, tag="w_n")
            nc.scalar.dma_start(wv_r[:], wv_d[0, 128 * cj:128 * (cj + 1), :])
            nc.gpsimd.dma_start(wv_i[:], wv_d[1, 128 * cj:128 * (cj + 1), :])
            nc.sync.dma_start(wv_n[:], wv_d[2, 128 * cj:128 * (cj + 1), :])
            st = cj == 0
            for kc in kcg:
                k0, szk = K_CHUNKS[kc]
                pr, pi = pps[kc]
                nc.tensor.matmul(pr[:szk, :256], _r(xnTr[:, cj, k0:k0 + szk]),
                                 _r(wv_r[:]), start=st, stop=False)
                nc.tensor.matmul(pr[:szk, :256], _r(xnTi[:, cj, k0:k0 + szk]),
                                 _r(wv_n[:]), start=False, stop=False)
                nc.tensor.matmul(pi[:szk, :256], _r(xnTr[:, cj, k0:k0 + szk]),
                                 _r(wv_i[:]), start=st, stop=False)
                nc.tensor.matmul(pi[:szk, :256], _r(xnTi[:, cj, k0:k0 + szk]),
                                 _r(wv_r[:]), start=False, stop=False)
        for kc in kcg:
            k0, szk = K_CHUNKS[kc]
            pr, pi = pps[kc]
            nc.tensor.matmul(pr[:szk, :256], _r(ones_row[:, :szk]), _r(bv_re[:]),
                             start=False, stop=True)
            nc.tensor.matmul(pi[:szk, :256], _r(ones_row[:, :szk]), _r(bv_im[:]),
                             start=False, stop=True)
            vr_v = pr[:szk, :256].rearrange("p (h d) -> p h d", h=4)
            vi_v = pi[:szk, :256].rearrange("p (h d) -> p h d", h=4)
            nc.vector.tensor_copy(vpk[:szk, kc, :, 0:64], vr_v)
            nc.vector.tensor_copy(vpk[:szk, kc, :, 64:128], vi_v)
    # kc4 (szk=64) packs head pairs along partitions in phase C: odd heads
    # read their v from rows 64:128
    nc.sync.dma_start(vpk[64:128, 4, 1, :], vpk[0:64, 4, 1, :])
    nc.gpsimd.dma_start(vpk[64:128, 4, 3, :], vpk[0:64, 4, 3, :])

    if DEBUG:
        for cj in range(4):
            nc.sync.dma_start(dbg["xnT"][0, 128 * cj:128 * (cj + 1), :],
                              xnTr[:, cj, :].bitcast(F32))
            nc.sync.dma_start(dbg["xnT"][1, 128 * cj:128 * (cj + 1), :],
                              xnTi[:, cj, :].bitcast(F32))
        nc.sync.dma_start(dbg["kA"][:], kA.bitcast(F32))
        nc.sync.dma_start(dbg["kB"][:], kB.bitcast(F32))

    bctx.close()

    # =====================================================================
    # Phase C: per q-chunk: q-proj -> scores -> softmax -> attn@v -> proj
    # =====================================================================
    cctx = contextlib.ExitStack()
    sm = cctx.enter_context(tc.tile_pool(name="sm", bufs=2))
    psC = cctx.enter_context(tc.tile_pool(name="psC", bufs=4, space="PSUM"))
    psOp = cctx.enter_context(tc.tile_pool(name="psOp", bufs=2, space="PSUM"))
    psDn = cctx.enter_context(tc.tile_pool(name="psDn", bufs=2, space="PSUM"))

    def emit_front(q0, nq, hp, qcs):
        """Scores + softmax numerators for head-pair hp; returns ebufs.

        kc 0-3: [szk, 2, nq] tiles (head i in dim 1).  kc 4 (szk=64):
        both heads packed in the partition dim (i0 rows 0:64, i1 64:128)
        so every elementwise op runs one [128, nq] pass instead of two.
        """
        sts = []
        for kc in range(4):
            k0, szk = K_CHUNKS[kc]
            s_t = sm.tile([128, 2, 512], F32, tag="s_t", bufs=4,
                          name=f"s{kc}")
            for i in range(2):
                h = 2 * hp + i
                qc = qcs[h]
                sre = psC.tile([128, 512], F32, tag="bank", name="sre")
                sim = psC.tile([128, 512], F32, tag="bank", name="sim")
                nc.tensor.matmul(sre[:szk, :nq], kA[:, h, k0:k0 + szk],
                                 qc[:, :nq], start=True, stop=True)
                nc.tensor.matmul(sim[:szk, :nq], kB[:, h, k0:k0 + szk],
                                 qc[:, :nq], start=True, stop=True)
                # s = sre^2 + sim^2 in f32
                nc.scalar.activation(s_t[:szk, i, :nq], sre[:szk, :nq],
                                     AF.Square)
                tmp = sm.tile([128, 512], F32, tag="tmp", bufs=2)
                if SQMODE == 0:
                    tmc = sm.tile([128, 512], F32, tag="tmc", bufs=2)
                    nc.vector.tensor_copy(tmc[:szk, :nq], sim[:szk, :nq])
                    nc.vector.tensor_mul(tmp[:szk, :nq], tmc[:szk, :nq],
                                         tmc[:szk, :nq])
                    nc.gpsimd.tensor_add(s_t[:szk, i, :nq], s_t[:szk, i, :nq],
                                         tmp[:szk, :nq])
                else:
                    nc.scalar.activation(tmp[:szk, :nq], sim[:szk, :nq],
                                         AF.Square)
                    nc.vector.tensor_add(s_t[:szk, i, :nq], s_t[:szk, i, :nq],
                                         tmp[:szk, :nq])
            if DEBUG and q0 == 0 and hp == 0 and kc < 2:
                nc.sync.dma_start(dbg["s"][kc, :szk, :, :nq], s_t[:szk, :, :nq])
            sts.append(s_t)
        # kc 4: head-pair packed [128, nq]
        s4 = sm.tile([128, 512], F32, tag="s4", bufs=3)
        sre4 = psC.tile([128, 512], F32, tag="bank", name="sre4")
        sim4 = psC.tile([128, 512], F32, tag="bank", name="sim4")
        for i in range(2):
            h = 2 * hp + i
            # i=0: cols [576:704) -> kc4 data lands in out rows 0:64;
            # i=1: cols [512:640) -> rows 64:128; zero cols pad the rest.
            c0 = 576 - 64 * i
            nc.tensor.matmul(sre4[:, :nq], kA[:, h, c0:c0 + 128],
                             qcs[h][:, :nq], start=i == 0, stop=i == 1)
            nc.tensor.matmul(sim4[:, :nq], kB[:, h, c0:c0 + 128],
                             qcs[h][:, :nq], start=i == 0, stop=i == 1)
        nc.scalar.activation(s4[:, :nq], sre4[:, :nq], AF.Square)
        tmp4 = sm.tile([128, 512], F32, tag="tmp", bufs=2)
        if SQMODE == 0:
            tmc4 = sm.tile([128, 512], F32, tag="tmc", bufs=2)
            nc.vector.tensor_copy(tmc4[:, :nq], sim4[:, :nq])
            nc.vector.tensor_mul(tmp4[:, :nq], tmc4[:, :nq], tmc4[:, :nq])
            nc.gpsimd.tensor_add(s4[:, :nq], s4[:, :nq], tmp4[:, :nq])
        else:
            nc.scalar.activation(tmp4[:, :nq], sim4[:, :nq], AF.Square)
            nc.vector.tensor_add(s4[:, :nq], s4[:, :nq], tmp4[:, :nq])
        # batched LUT runs: |a| = sqrt(s) (or exp(0.5 ln s)), then
        # ebuf = exp(|a| - 50) in bf16.  Softmax is shift-invariant so the
        # constant -50 cancels; it just keeps exp sums in range.
        if SQRTMODE == 0:
            for kc in range(4):
                szk = K_CHUNKS[kc][1]
                nc.scalar.activation(sts[kc][:szk, :, :nq],
                                     sts[kc][:szk, :, :nq], AF.Ln)
            nc.scalar.activation(s4[:, :nq], s4[:, :nq], AF.Ln)
            for kc in range(4):
                szk = K_CHUNKS[kc][1]
                nc.scalar.activation(sts[kc][:szk, :, :nq],
                                     sts[kc][:szk, :, :nq], AF.Exp, scale=0.5)
            nc.scalar.activation(s4[:, :nq], s4[:, :nq], AF.Exp, scale=0.5)
        else:
            for kc in range(4):
                szk = K_CHUNKS[kc][1]
                nc.scalar.activation(sts[kc][:szk, :, :nq],
                                     sts[kc][:szk, :, :nq], AF.Sqrt)
            nc.scalar.activation(s4[:, :nq], s4[:, :nq], AF.Sqrt)
        ebufs = []
        for kc in range(4):
            szk = K_CHUNKS[kc][1]
            ebuf = sm.tile([128, 2, 512], BF16, tag="ebuf", bufs=8,
                           name=f"eb{kc}")
            nc.scalar.activation(ebuf[:szk, :, :nq], sts[kc][:szk, :, :nq],
                                 AF.Exp, bias=nbias[:szk])
            ebufs.append(ebuf)
        eb4 = sm.tile([128, 512], BF16, tag="eb4", bufs=3)
        nc.scalar.activation(eb4[:, :nq], s4[:, :nq], AF.Exp, bias=nbias[:])
        ebufs.append(eb4)
        return ebufs

    def emit_back(q0, nq, hp, ebufs):
        """attn@v + denominators + normalize for head-pair hp."""
        ops = [psOp.tile([128, 512], F32, tag="op", name=f"op{i}")
               for i in range(2)]
        dnp = psDn.tile([128, 512], F32, tag="dn", name="dnp")
        for kc in range(4):
            k0, szk = K_CHUNKS[kc]
            ebuf = ebufs[kc]
            for i in range(2):
                h = 2 * hp + i
                nc.tensor.matmul(ops[i][:, :nq], vpk[:szk, kc, h, :],
                                 ebuf[:szk, i, :nq], start=kc == 0, stop=False)
                nc.tensor.matmul(dnp[32 * i:32 * i + 1, :nq], ones_col[:szk, :],
                                 ebuf[:szk, i, :nq], start=kc == 0, stop=False,
                                 tile_position=(0, 32 * i))
        eb4 = ebufs[4]
        for i in range(2):
            h = 2 * hp + i
            rs = slice(64 * i, 64 * (i + 1))
            mask = ones_h0 if i == 0 else ones_h1
            nc.tensor.matmul(ops[i][:, :nq], vpk[rs, 4, h, :],
                             eb4[rs, :nq], start=False, stop=True)
            nc.tensor.matmul(dnp[32 * i:32 * i + 1, :nq], mask[:, :],
                             eb4[:, :nq], start=False, stop=True,
                             tile_position=(0, 32 * i))
        dnt = sm.tile([1, 1024], F32, tag="dnt", bufs=2)
        dns = sm.tile([1, 1024], F32, tag="dns", bufs=2)
        dnr = sm.tile([1, 1024], F32R, tag="dnr", bufs=2)
        for i in range(2):
            c = slice(512 * i, 512 * i + nq)
            # ACT partition-shifts row 32i of PSUM into partition 0;
            # reciprocal_approx_fast is broken for base_partition != 0 on HW
            nc.scalar.copy(dnt[0:1, c], dnp[32 * i:32 * i + 1, :nq])
            if RECIPMODE == 0:
                nc.vector.reciprocal_approx_fast(dns[0:1, c], dnt[0:1, c])
                nc.vector.tensor_copy(dnr[0:1, c], dns[0:1, c])
            else:
                nc.scalar.activation(dns[0:1, c], dnt[0:1, c], AF.Ln)
                nc.scalar.activation(dnr[0:1, c], dns[0:1, c], AF.Exp,
                                     scale=-1.0)
        oris = []
        for i in range(2):
            osb = sm.tile([128, 512], F32, tag="osb", bufs=2, name=f"osb{i}")
            nc.scalar.copy(osb[:, :nq], ops[i][:, :nq])
            rbp = psC.tile([128, 512], F32, tag="bank", name="rbp")
            nc.tensor.matmul(rbp[:, :nq], _r(ones128[0:1, :]),
                             dnr[0:1, 512 * i:512 * i + nq],
                             start=True, stop=True)
            ori = sm.tile([128, 512], F16, tag="ori", bufs=5, name=f"ori{i}")
            nc.vector.tensor_mul(ori[:, :nq], osb[:, :nq], rbp[:, :nq])
            oris.append(ori)
        return oris

    def emit_proj(q0, nq, oris):
        """Partial output projection for one chunk; oris = [h0..h3]."""
        for cc in range(4):
            cs = slice(128 * cc, 128 * (cc + 1))
            pr = psC.tile([128, 512], F32, tag="bank", name="pjr")
            pi = psC.tile([128, 512], F32, tag="bank", name="pji")
            for h in range(4):
                st = h == 0
                sp = h == 3
                nc.tensor.matmul(pr[:, :nq], wps[:, 0, h, cs], oris[h][:, :nq],
                                 start=st, stop=sp)
                nc.tensor.matmul(pi[:, :nq], wps[:, 1, h, cs], oris[h][:, :nq],
                                 start=st, stop=sp)
            o1 = sm.tile([128, 512], F16, tag="o1", bufs=2)
            o2 = sm.tile([128, 512], F16, tag="o2", bufs=2)
            nc.vector.tensor_copy(o1[:, :nq], pr[:, :nq])
            nc.vector.tensor_copy(o2[:, :nq], pi[:, :nq])
            nc.gpsimd.dma_start(outT_d[0, cs, q0:q0 + nq], o1[:, :nq])
            nc.sync.dma_start(outT_d[1, cs, q0:q0 + nq], o2[:, :nq])

    # software pipeline, two half-steps deep: back(H[i-2]) is emitted after
    # front(H[i]) so its ebufs have had two full half-steps of ACT/DVE time;
    # the PE meanwhile streams sc(H[i]) + av(H[i-2]) + qp/proj with no stall.
    halves = [(q0, nq, hp) for (q0, nq) in Q_CHUNKS for hp in range(2)]
    qcs_by_q0 = dict(qcs_early)
    fronts = []
    ori_by_q0 = {}

    def _drain(idx):
        bq0, bnq, bhp, bebufs = fronts[idx]
        oris = emit_back(bq0, bnq, bhp, bebufs)
        ori_by_q0.setdefault(bq0, []).extend(oris)
        if bhp == 1:
            emit_proj(bq0, bnq, ori_by_q0.pop(bq0))

    for i, (q0, nq, hp) in enumerate(halves):
        ebufs = emit_front(q0, nq, hp, qcs_by_q0[q0])
        fronts.append((q0, nq, hp, ebufs))
        qi = i // 2
        if hp == 1 and qi + 2 < len(Q_CHUNKS):
            nq0, nnq = Q_CHUNKS[qi + 2]
            qcs_by_q0[nq0] = emit_qp(nq0, nnq, psC)
        if i >= 2:
            _drain(i - 2)
    _drain(len(halves) - 2)
    _drain(len(halves) - 1)

    cctx.close()
    ctx.close()


# =========================================================================
# Host side
# =========================================================================

def _f32(x):
    return np.ascontiguousarray(x, dtype=np.float32)


def _f16(x):
    return np.ascontiguousarray(np.asarray(x, dtype=np.float16))


def host_prep(x_re, x_im, Wq, Wkv, Wproj, bproj, sr_w, sr_b, gain, bias):
    x_re = np.asarray(x_re)
    x_im = np.asarray(x_im)
    Wq = np.asarray(Wq)
    Wkv = np.asarray(Wkv)
    Wproj = np.asarray(Wproj)
    sr_w = np.asarray(sr_w)
    sr_b = np.asarray(sr_b)
    gain = np.asarray(gain)
    bias = np.asarray(bias)

    Wkv_eff = gain[:, None] * Wkv
    bkv_full = bias @ Wkv
    Wc = sr_w.transpose(2, 3, 1, 0).reshape(4 * C, C)

    def planes3f(w):
        return np.stack([_f32(w.real), _f32(w.imag), _f32(-w.imag)])

    in_maps = []
    for core in range(8):
        b, g = core // 2, core % 2
        cols = slice(256 * g, 256 * (g + 1))
        wk_c = Wkv_eff[:, :C][:, cols] * SCALE
        wv_c = Wkv_eff[:, C:][:, cols]
        bk_c = bkv_full[:C][cols] * SCALE
        bv_c = bkv_full[C:][cols]
        xs_c = np.stack([x_re[b].T, x_im[b].T])  # [2, C, N]
        xsp = xs_c.reshape(2, C, HR, 2, HR, 2)
        xP = np.stack([xsp[:, :, :, p, :, q].reshape(2, C, NK)
                       for p in range(2) for q in range(2)], axis=1)
        # wpA_h = [Wp_re rows of head h ; -Wp_im rows], wpB_h = [im ; re]
        wp_blk = Wproj[256 * g:256 * (g + 1), :]  # [256, C] complex
        wpab = np.zeros((2, 4, 128, C), np.float16)
        for h in range(4):
            rows = wp_blk[64 * h:64 * (h + 1), :]
            wpab[0, h, 0:64] = _f16(rows.real)
            wpab[0, h, 64:128] = _f16(-rows.imag)
            wpab[1, h, 0:64] = _f16(rows.imag)
            wpab[1, h, 64:128] = _f16(rows.real)
        m = {
            "xT": _f32(xs_c),
            "xP": _f32(xP.reshape(2, 4 * C, NK)),
            "wc": planes3f(Wc),
            "srb": np.stack([_f32(sr_b.real), _f32(sr_b.imag)]),
            "ones": np.ones((1, 512), np.float32),
            "wq": planes3f(Wq[:, cols]),
            "wk": planes3f(wk_c),
            "wv": planes3f(wv_c),
            "wpab": wpab,
            "bkv": np.stack([
                np.stack([_f32(bk_c.real), _f32(bv_c.real)]),
                np.stack([_f32(bk_c.imag), _f32(bv_c.imag)]),
            ]),
        }
        in_maps.append(m)
    return in_maps


_NC_CACHE = None


def _get_nc():
    global _NC_CACHE
    if _NC_CACHE is None:
        _NC_CACHE = build_nc()
    return _NC_CACHE


def kernel(x_re, x_im, Wq, Wkv, Wproj, bproj, sr_w, sr_b, gain, bias, H, W):
    from concourse.bass_utils import run_bass_kernel_spmd

    nc = _get_nc()
    in_maps = host_prep(x_re, x_im, Wq, Wkv, Wproj, bproj, sr_w, sr_b, gain, bias)
    res = run_bass_kernel_spmd(nc, in_maps, list(range(8)))
    bproj = np.asarray(bproj)
    out = np.zeros((B, N, C), dtype=np.complex64)
    for b in range(B):
        p0 = res.results[2 * b]["outT"].astype(np.float32)
        p1 = res.results[2 * b + 1]["outT"].astype(np.float32)
        acc = (p0[0] + p1[0]).T + 1j * (p0[1] + p1[1]).T
        out[b] = acc + bproj[None, :]
    return out


# revision 25
# speedup vs baseline: 1.2005x; 1.0565x over previous
"""Trainium2 Bass kernel for complex-valued spatial-reduction attention.

x: [B=4, N=2304, C=512] complex64 (re/im f32 planes), H=W=48, 8 heads,
head_dim 64, sr_ratio 2 -> Nk=576.

Sharding: 8 cores = 4 batches x 2 head-groups (4 heads each). Each core:
sr-conv over full C, complex LayerNorm, q/k/v for its heads,
softmax(|q.k^T|) attention, attn @ v, partial output projection.
Host sums the two partials per batch and adds bproj.

v2 structure: scores use K=128 packing (kA=[kr;-ki], kB=[ki;kr],
qcat=[qr;qi]); q-projection and output projection run inside the
attention loop (q/attn-out never round-trip DRAM); softmax runs
Square(ACT f32) + sim^2/add (DVE f32) + batched Ln/Exp runs;
denominator reciprocal on DVE (no act-table thrash); v/attn/proj in f16.
"""

import os
import contextlib

import numpy as np
import ml_dtypes

import concourse.bass as bass
import concourse.mybir as mybir
import concourse.tile as tile
from concourse import bacc
from concourse.masks import make_identity

BF16 = mybir.dt.bfloat16
F16 = mybir.dt.float16
F32 = mybir.dt.float32
F32R = mybir.dt.float32r
AF = mybir.ActivationFunctionType
ALU = mybir.AluOpType

B, N, C, HEADS, HD, SR = 4, 2304, 512, 8, 64, 2
NK = 576
HR = 24
EPS = 1e-5
SCALE = HD ** -0.5  # folded into Wk host-side

M_GROUPS = [
    [(0, 5), (5, 5), (10, 5)],
    [(15, 5), (20, 4)],
]
K_CHUNKS = [(0, 128), (128, 128), (256, 128), (384, 128), (512, 64)]
Q_CHUNKS = [(0, 512), (512, 512), (1024, 512), (1536, 512), (2048, 256)]

# 0 = ln/exp sqrt (safe), 1 = ACT Sqrt LUT (faster; precision probe)
SQRTMODE = int(os.environ.get("KBUILD_SQRT", "1"))
# 0 = DVE tensor_mul(sim, sim) reading same PSUM AP twice; 1 = ACT Square
SQMODE = int(os.environ.get("KBUILD_SQ", "0"))
DEBUG = bool(int(os.environ.get("KBUILD_DEBUG", "0")))
# 1 = baseline ln/exp reciprocal instead of reciprocal_approx_fast
RECIPMODE = int(os.environ.get("KBUILD_RECIP", "0"))


def _r(ap):
    return ap.bitcast(F32R)


def build_nc():
    nc = bacc.Bacc("TRN2", target_bir_lowering=False, debug=False, num_devices=8)

    xT_d = nc.dram_tensor("xT", [2, C, N], F32R, kind="ExternalInput")
    xP_d = nc.dram_tensor("xP", [2, 4 * C, NK], F32R, kind="ExternalInput")
    wc_d = nc.dram_tensor("wc", [3, 4 * C, C], F32R, kind="ExternalInput")
    srb_d = nc.dram_tensor("srb", [2, C], F32R, kind="ExternalInput")
    ones_d = nc.dram_tensor("ones", [1, 512], F32R, kind="ExternalInput")
    wq_d = nc.dram_tensor("wq", [3, C, 256], F32R, kind="ExternalInput")
    wk_d = nc.dram_tensor("wk", [3, C, 256], F32R, kind="ExternalInput")
    wv_d = nc.dram_tensor("wv", [3, C, 256], F32R, kind="ExternalInput")
    wpab_d = nc.dram_tensor("wpab", [2, 4, 128, C], F16, kind="ExternalInput")
    bkv_d = nc.dram_tensor("bkv", [2, 2, 256], F32R, kind="ExternalInput")
    outT_d = nc.dram_tensor("outT", [2, C, N], F16, kind="ExternalOutput")
    dbg = {}
    if DEBUG:
        dbg["kA"] = nc.dram_tensor("dbg_kA", [128, 4, NK], F32, kind="ExternalOutput")
        dbg["kB"] = nc.dram_tensor("dbg_kB", [128, 4, NK], F32, kind="ExternalOutput")
        dbg["qc"] = nc.dram_tensor("dbg_qc", [4, 128, 512], F32, kind="ExternalOutput")
        dbg["s"] = nc.dram_tensor("dbg_s", [2, 128, 2, 512], F32, kind="ExternalOutput")
        dbg["xnT"] = nc.dram_tensor("dbg_xnT", [2, C, NK], F32, kind="ExternalOutput")

    with tile.TileContext(nc) as tc:
        _body(nc, tc, xT_d, xP_d, wc_d, srb_d, ones_d, wq_d, wk_d, wv_d,
              wpab_d, bkv_d, outT_d, dbg)

    nc.compile()
    return nc


def _ln_chunk(nc, work, stats, cre, cim, sz):
    """Complex LayerNorm for one [sz, C] chunk in PSUM -> (xnr, xni)."""
    inv_c = 1.0 / C
    re_sb = work.tile([128, C], F32, tag="ln_re")
    im_sb = work.tile([128, C], F32, tag="ln_im")
    sum_r = stats.tile([128, 1], F32, tag="sum_r")
    sum_i = stats.tile([128, 1], F32, tag="sum_i")
    nc.vector.tensor_copy(re_sb[:sz], cre[:sz, :])
    nc.vector.tensor_copy(im_sb[:sz], cim[:sz, :])
    nc.vector.tensor_reduce(sum_r[:sz], re_sb[:sz], mybir.AxisListType.X, ALU.add)
    nc.vector.tensor_reduce(sum_i[:sz], im_sb[:sz], mybir.AxisListType.X, ALU.add)
    junk = work.tile([128, C], F32, tag="ln_junk", bufs=1)
    sxx = stats.tile([128, 1], F32, tag="sxx")
    sii = stats.tile([128, 1], F32, tag="sii")
    sxi = stats.tile([128, 1], F32, tag="sxi")
    nc.vector.tensor_mul(junk[:sz], re_sb[:sz], re_sb[:sz])
    nc.vector.tensor_reduce(sxx[:sz], junk[:sz], mybir.AxisListType.X, ALU.add)
    nc.vector.tensor_mul(junk[:sz], im_sb[:sz], im_sb[:sz])
    nc.vector.tensor_reduce(sii[:sz], junk[:sz], mybir.AxisListType.X, ALU.add)
    nc.vector.tensor_mul(junk[:sz], re_sb[:sz], im_sb[:sz])
    nc.vector.tensor_reduce(sxi[:sz], junk[:sz], mybir.AxisListType.X, ALU.add)
    mr = stats.tile([128, 1], F32, tag="mr")
    mi = stats.tile([128, 1], F32, tag="mi")
    nc.vector.tensor_scalar_mul(mr[:sz], sum_r[:sz], inv_c)
    nc.vector.tensor_scalar_mul(mi[:sz], sum_i[:sz], inv_c)
    vre = stats.tile([128, 1], F32, tag="vre")
    vim = stats.tile([128, 1], F32, tag="vim")
    tA = stats.tile([128, 1], F32, tag="tA")
    tB = stats.tile([128, 1], F32, tag="tB")
    nc.vector.tensor_sub(tA[:sz], sxx[:sz], sii[:sz])
    nc.vector.tensor_scalar_mul(tA[:sz], tA[:sz], inv_c)
    nc.vector.tensor_mul(vre[:sz], mr[:sz], mr[:sz])
    nc.vector.tensor_mul(tB[:sz], mi[:sz], mi[:sz])
    nc.vector.tensor_sub(vre[:sz], vre[:sz], tB[:sz])
    nc.vector.tensor_sub(vre[:sz], tA[:sz], vre[:sz])
    nc.vector.tensor_scalar_add(vre[:sz], vre[:sz], EPS)
    nc.vector.tensor_mul(tB[:sz], mr[:sz], mi[:sz])
    nc.vector.tensor_scalar_mul(tB[:sz], tB[:sz], 2.0)
    nc.vector.tensor_scalar_mul(vim[:sz], sxi[:sz], 2.0 * inv_c)
    nc.vector.tensor_sub(vim[:sz], vim[:sz], tB[:sz])
    r2 = stats.tile([128, 1], F32, tag="r2")
    nc.vector.tensor_mul(r2[:sz], vre[:sz], vre[:sz])
    nc.vector.tensor_mul(tB[:sz], vim[:sz], vim[:sz])
    nc.vector.tensor_add(r2[:sz], r2[:sz], tB[:sz])

    def _sqrt_newton(out, x, sc):
        # y0 = LUT sqrt(sc*x); y1 = 0.5*(y0 + sc*x/y0)  (one Newton step)
        y0 = stats.tile([128, 1], F32, tag="nw_y0")
        nc.scalar.activation(y0[:sz], x[:sz], AF.Sqrt, scale=sc)
        yr = stats.tile([128, 1], F32, tag="nw_yr")
        nc.vector.tensor_scalar_add(y0[:sz], y0[:sz], 1e-30)
        nc.vector.reciprocal(yr[:sz], y0[:sz])
        nc.vector.tensor_mul(yr[:sz], yr[:sz], x[:sz])
        if sc != 1.0:
            nc.vector.tensor_scalar_mul(yr[:sz], yr[:sz], sc)
        nc.vector.tensor_add(out[:sz], y0[:sz], yr[:sz])
        nc.vector.tensor_scalar_mul(out[:sz], out[:sz], 0.5)

    rr = stats.tile([128, 1], F32, tag="rr")
    _sqrt_newton(rr, r2, 1.0)
    srt = stats.tile([128, 1], F32, tag="srt")
    sia = stats.tile([128, 1], F32, tag="sia")
    nc.vector.tensor_add(tA[:sz], rr[:sz], vre[:sz])
    _sqrt_newton(srt, tA, 0.5)
    nc.vector.tensor_sub(tA[:sz], rr[:sz], vre[:sz])
    _sqrt_newton(sia, tA, 0.5)
    sgn = stats.tile([128, 1], F32, tag="sgn")
    nc.scalar.activation(sgn[:sz], vim[:sz], AF.Sign)
    nc.vector.tensor_mul(sia[:sz], sia[:sz], sgn[:sz])
    rin = stats.tile([128, 1], F32, tag="rin")
    nc.vector.reciprocal(rin[:sz], rr[:sz])
    wr = stats.tile([128, 1], F32, tag="wr")
    wn = stats.tile([128, 1], F32, tag="wn")  # = -w_im
    nc.vector.tensor_mul(wr[:sz], srt[:sz], rin[:sz])
    nc.vector.tensor_mul(wn[:sz], sia[:sz], rin[:sz])
    aT = work.tile([128, C], F32, tag="ln_a")
    bT = work.tile([128, C], F32, tag="ln_b")
    xnr = work.tile([128, C], F32, tag="ln_xnr")
    xni = work.tile([128, C], F32, tag="ln_xni")
    nc.vector.tensor_scalar(aT[:sz], re_sb[:sz], mr[:sz], wr[:sz],
                            ALU.subtract, ALU.mult)
    nc.vector.tensor_scalar(bT[:sz], im_sb[:sz], mi[:sz], wn[:sz],
                            ALU.subtract, ALU.mult)
    nc.vector.tensor_add(xnr[:sz], aT[:sz], bT[:sz])
    nc.vector.tensor_scalar(aT[:sz], re_sb[:sz], mr[:sz], wn[:sz],
                            ALU.subtract, ALU.mult)
    nc.vector.tensor_scalar(bT[:sz], im_sb[:sz], mi[:sz], wr[:sz],
                            ALU.subtract, ALU.mult)
    nc.vector.tensor_sub(xni[:sz], bT[:sz], aT[:sz])
    return xnr, xni


def _body(nc, tc, xT_d, xP_d, wc_d, srb_d, ones_d, wq_d, wk_d, wv_d,
          wpab_d, bkv_d, outT_d, dbg=None):
    ctx = contextlib.ExitStack()
    consts = ctx.enter_context(tc.tile_pool(name="consts", bufs=1))
    big = ctx.enter_context(tc.tile_pool(name="big", bufs=1))
    xqp = ctx.enter_context(tc.tile_pool(name="xqp", bufs=4))
    qd = ctx.enter_context(tc.tile_pool(name="qd", bufs=4))
    qcp = ctx.enter_context(tc.tile_pool(name="qcp", bufs=8))

    # ---- constants ----
    ident = consts.tile([128, 128], F32, tag="ident")
    make_identity(nc, ident)
    ones_col = consts.tile([128, 1], BF16, tag="ones_col")
    nc.vector.memset(ones_col, 1.0)
    ones_h0 = consts.tile([128, 1], BF16, tag="ones_h0")
    nc.vector.memset(ones_h0, 0.0)
    nc.vector.memset(ones_h0[0:64], 1.0)
    ones_h1 = consts.tile([128, 1], BF16, tag="ones_h1")
    nc.vector.memset(ones_h1, 0.0)
    nc.vector.memset(ones_h1[64:128], 1.0)
    ones128 = consts.tile([128, 128], F32, tag="ones128")
    nc.vector.memset(ones128, 1.0)
    ones_row = consts.tile([1, 512], F32R, tag="ones_row")
    nc.sync.dma_start(ones_row[:], ones_d[:])
    nbias = consts.tile([128, 1], F32, tag="nbias")
    nc.vector.memset(nbias, -50.0)

    srb_re = consts.tile([1, C], F32R, tag="srb_re")
    srb_im = consts.tile([1, C], F32R, tag="srb_im")
    nc.sync.dma_start(srb_re[:], srb_d[0:1, :])
    nc.sync.dma_start(srb_im[:], srb_d[1:2, :])
    bk_re = consts.tile([1, 256], F32R, tag="bk_re")
    bk_im = consts.tile([1, 256], F32R, tag="bk_im")
    bv_re = consts.tile([1, 256], F32R, tag="bv_re")
    bv_im = consts.tile([1, 256], F32R, tag="bv_im")
    nc.sync.dma_start(bk_re[:], bkv_d[0:1, 0, :])
    nc.sync.dma_start(bv_re[:], bkv_d[0:1, 1, :])
    nc.sync.dma_start(bk_im[:], bkv_d[1:2, 0, :])
    nc.sync.dma_start(bv_im[:], bkv_d[1:2, 1, :])

    # ---- persistent SBUF ----
    xnTr = big.tile([128, 4, NK], F32R, tag="xnTr")
    xnTi = big.tile([128, 4, NK], F32R, tag="xnTi")
    # kA_h = [kr_h ; -ki_h], kB_h = [ki_h ; kr_h]  (partition dim 64+64)
    kA = big.tile([128, 4, 704], F32R, tag="kA")
    kB = big.tile([128, 4, 704], F32R, tag="kB")
    nc.vector.memset(kA.bitcast(F32)[:, :, 512:576], 0.0)
    nc.vector.memset(kA.bitcast(F32)[:, :, 640:704], 0.0)
    nc.vector.memset(kB.bitcast(F32)[:, :, 512:576], 0.0)
    nc.vector.memset(kB.bitcast(F32)[:, :, 640:704], 0.0)
    vpk = big.tile([128, 5, 4, 128], BF16, tag="vpk")
    wqs = big.tile([128, 3, 4, 256], F32R, tag="wqs")
    nc.sync.dma_start(wqs[:], wq_d.rearrange("s (j p) n -> p s j n", p=128))
    wps = big.tile([128, 2, 4, C], F16, tag="wps")
    nc.sync.dma_start(wps[:], wpab_d.rearrange("a h p n -> p a h n"))

    xT_v = [xT_d[pl].rearrange("(j p) t -> p j t", p=128) for pl in (0, 1)]

    # =====================================================================
    # Phase B: conv (f32r) + LayerNorm + transposes + k/v projections
    # =====================================================================
    bctx = contextlib.ExitStack()
    xs = bctx.enter_context(tc.tile_pool(name="xs", bufs=4))
    wcp = bctx.enter_context(tc.tile_pool(name="wcp", bufs=2))
    ws = bctx.enter_context(tc.tile_pool(name="ws", bufs=2))
    work = bctx.enter_context(tc.tile_pool(name="work", bufs=2))
    stats = bctx.enter_context(tc.tile_pool(name="stats", bufs=2))
    psum = bctx.enter_context(tc.tile_pool(name="psumB", bufs=7, space="PSUM"))

    def emit_qp(q0, nq, pspool):
        """q-projection for one chunk; returns qc[h] = [qr_h;qi_h] tiles."""
        prs = []
        for half in range(2):
            prs.append((pspool.tile([128, 512], F32, tag="bank", name=f"qpr{half}"),
                        pspool.tile([128, 512], F32, tag="bank", name=f"qpi{half}")))
        for cj in range(4):
            xq_r = xqp.tile([128, 512], F32R, tag="xq_r")
            xq_i = xqp.tile([128, 512], F32R, tag="xq_i")
            nc.gpsimd.dma_start(xq_r[:, :nq], xT_v[0][:, cj, q0:q0 + nq])
            nc.sync.dma_start(xq_i[:, :nq], xT_v[1][:, cj, q0:q0 + nq])
            st = cj == 0
            sp = cj == 3
            for half in range(2):
                hs = slice(128 * half, 128 * (half + 1))
                pr, pi = prs[half]
                nc.tensor.matmul(pr[:, :nq], wqs[:, 0, cj, hs], _r(xq_r[:, :nq]),
                                 start=st, stop=False)
                nc.tensor.matmul(pr[:, :nq], wqs[:, 2, cj, hs], _r(xq_i[:, :nq]),
                                 start=False, stop=sp)
                nc.tensor.matmul(pi[:, :nq], wqs[:, 1, cj, hs], _r(xq_r[:, :nq]),
                                 start=st, stop=False)
                nc.tensor.matmul(pi[:, :nq], wqs[:, 0, cj, hs], _r(xq_i[:, :nq]),
                                 start=False, stop=sp)
        qcs = []
        for half in range(2):
            pr, pi = prs[half]
            qr_sb = qd.tile([128, 512], F32R, tag="qd", name=f"qr{half}")
            qi_sb = qd.tile([128, 512], F32R, tag="qd", name=f"qi{half}")
            nc.scalar.copy(qr_sb[:, :nq], pr[:, :nq])
            nc.scalar.copy(qi_sb[:, :nq], pi[:, :nq])
            for j in range(2):
                h = 2 * half + j
                qc = qcp.tile([128, 512], F32R, tag="qc", name=f"qc{h}")
                rs = slice(64 * j, 64 * (j + 1))
                nc.sync.dma_start(qc[0:64, :nq], qr_sb[rs, :nq])
                nc.gpsimd.dma_start(qc[64:128, :nq], qi_sb[rs, :nq])
                if DEBUG and q0 == 0:
                    nc.sync.dma_start(dbg["qc"][h, :, :nq], qc[:, :nq].bitcast(F32))
                qcs.append(qc)
        return qcs


    qcs_early = {}

    for mg in M_GROUPS:
        tg0 = mg[0][0] * HR
        tgs = sum(nh for _, nh in mg) * HR
        cps = []
        for (hr0, nh) in mg:
            cre = psum.tile([128, C], F32, tag="bank")
            cim = psum.tile([128, C], F32, tag="bank")
            cps.append((cre, cim, hr0 * HR - tg0, hr0 * HR, nh * HR))

        first = [[True, True] for _ in mg]
        for kk in range(16):
            xp_r = xs.tile([128, 3 * 120], F32R, tag="xp_r")
            xp_i = xs.tile([128, 3 * 120], F32R, tag="xp_i")
            nc.gpsimd.dma_start(xp_r[:, :tgs], xP_d[0, 128 * kk:128 * (kk + 1),
                                                    tg0:tg0 + tgs])
            nc.scalar.dma_start(xp_i[:, :tgs], xP_d[1, 128 * kk:128 * (kk + 1),
                                                  tg0:tg0 + tgs])
            w_re = wcp.tile([128, C], F32R, tag="wc_re")
            w_im = wcp.tile([128, C], F32R, tag="wc_im")
            w_in = wcp.tile([128, C], F32R, tag="wc_in")
            nc.scalar.dma_start(w_re[:], wc_d[0, 128 * kk:128 * (kk + 1), :])
            nc.sync.dma_start(w_im[:], wc_d[1, 128 * kk:128 * (kk + 1), :])
            nc.gpsimd.dma_start(w_in[:], wc_d[2, 128 * kk:128 * (kk + 1), :])
            for mi_, (cre, cim, off, t0, sz) in enumerate(cps):
                pat_r = xp_r[:, off:off + sz]
                pat_i = xp_i[:, off:off + sz]
                nc.tensor.matmul(cre[:sz, :], _r(pat_r), _r(w_re[:]),
                                 start=first[mi_][0], stop=False)
                nc.tensor.matmul(cim[:sz, :], _r(pat_r), _r(w_im[:]),
                                 start=first[mi_][1], stop=False)
                first[mi_] = [False, False]
                nc.tensor.matmul(cre[:sz, :], _r(pat_i), _r(w_in[:]),
                                 start=False, stop=False)
                nc.tensor.matmul(cim[:sz, :], _r(pat_i), _r(w_re[:]),
                                 start=False, stop=False)
        for (cre, cim, off, t0, sz) in cps:
            nc.tensor.matmul(cre[:sz, :], _r(ones_row[:, :sz]), _r(srb_re[:]),
                             start=False, stop=True)
            nc.tensor.matmul(cim[:sz, :], _r(ones_row[:, :sz]), _r(srb_im[:]),
                             start=False, stop=True)

        # q-projection for one early chunk fills the PE while the
        # LayerNorm chain below runs on DVE/ACT (shares the psum pool)
        mgi = 0 if mg[0][0] == 0 else 1
        qcs_early[Q_CHUNKS[mgi][0]] = emit_qp(*Q_CHUNKS[mgi], psum)

        # ---- LayerNorm + transpose into xnT ----
        for (cre, cim, off, t0, sz) in cps:
            xnr, xni = _ln_chunk(nc, work, stats, cre, cim, sz)
            for cj in range(4):
                for src, dst in ((xnr, xnTr), (xni, xnTi)):
                    pt = psum.tile([128, 128], F32, tag="bank")
                    nc.tensor.transpose(pt[:, :sz],
                                        src[:sz, 128 * cj:128 * (cj + 1)],
                                        ident[:sz, :sz])
                    nc.vector.tensor_copy(dst[:, cj, t0:t0 + sz], pt[:, :sz])

    # ---- k^T projection -> kA/kB (packed K=128 layout) ----
    # kn holds the negated / positive ki pieces that need partition shifts.
    kn = big.tile([128, 2, 704], F32R, tag="kn")
    nc.vector.memset(kn.bitcast(F32)[:, :, 512:576], 0.0)
    nc.vector.memset(kn.bitcast(F32)[:, :, 640:704], 0.0)
    for half in range(2):
        hs = slice(128 * half, 128 * (half + 1))
        p512r = psum.tile([128, 512], F32, tag="bank")
        p512i = psum.tile([128, 512], F32, tag="bank")
        p64r = psum.tile([128, 512], F32, tag="bank")
        p64i = psum.tile([128, 512], F32, tag="bank")
        for cj in range(4):
            wk_r = ws.tile([128, 256], F32R, tag="w_r")
            wk_i = ws.tile([128, 256], F32R, tag="w_i")
            wk_n = ws.tile([128, 256], F32R, tag="w_n")
            nc.scalar.dma_start(wk_r[:], wk_d[0, 128 * cj:128 * (cj + 1), :])
            nc.gpsimd.dma_start(wk_i[:], wk_d[1, 128 * cj:128 * (cj + 1), :])
            nc.sync.dma_start(wk_n[:], wk_d[2, 128 * cj:128 * (cj + 1), :])
            st = cj == 0
            for (pr, pi, n0, nn) in ((p512r, p512i, 0, 512), (p64r, p64i, 512, 64)):
                nc.tensor.matmul(pr[:, :nn], _r(wk_r[:, hs]),
                                 _r(xnTr[:, cj, n0:n0 + nn]), start=st, stop=False)
                nc.tensor.matmul(pr[:, :nn], _r(wk_n[:, hs]),
                                 _r(xnTi[:, cj, n0:n0 + nn]), start=False, stop=False)
                nc.tensor.matmul(pi[:, :nn], _r(wk_i[:, hs]),
                                 _r(xnTr[:, cj, n0:n0 + nn]), start=st, stop=False)
                nc.tensor.matmul(pi[:, :nn], _r(wk_r[:, hs]),
                                 _r(xnTi[:, cj, n0:n0 + nn]), start=False, stop=False)
        h0, h1 = 2 * half, 2 * half + 1
        for (pr, pi, n0, nn) in ((p512r, p512i, 0, 512), (p64r, p64i, 576, 64)):
            nc.tensor.matmul(pr[:, :nn], _r(bk_re[:, hs]), _r(ones_row[:, :nn]),
                             start=False, stop=True)
            nc.tensor.matmul(pi[:, :nn], _r(bk_im[:, hs]), _r(ones_row[:, :nn]),
                             start=False, stop=True)
            ns = slice(n0, n0 + nn)
            # aligned drains (no partition shift):
            nc.vector.tensor_copy(kA[0:64, h0, ns], pr[0:64, :nn])       # kr_h0
            nc.vector.tensor_copy(kB[64:128, h1, ns], pr[64:128, :nn])   # kr_h1
            nc.vector.tensor_copy(kB[0:64, h0, ns], pi[0:64, :nn])       # ki_h0
            nc.vector.tensor_scalar_mul(kA[64:128, h1, ns], pi[64:128, :nn],
                                        -1.0)                            # -ki_h1
            nc.vector.tensor_scalar_mul(kn[0:64, half, ns], pi[0:64, :nn],
                                        -1.0)                            # -ki_h0
            nc.vector.tensor_copy(kn[64:128, half, ns], pi[64:128, :nn])  # ki_h1
        # partition-shifting copies via SBUF->SBUF DMA
        nc.sync.dma_start(kA[64:128, h0, :], kn[0:64, half, :])     # -ki_h0
        nc.sync.dma_start(kB[0:64, h1, :], kn[64:128, half, :])     # ki_h1
        nc.gpsimd.dma_start(kA[0:64, h1, :], kB[64:128, h1, :])     # kr_h1
        nc.gpsimd.dma_start(kB[64:128, h0, :], kA[0:64, h0, :])     # kr_h0

    # ---- v projection (f32r) -> vpk f16 ----
    for kcg in ((0, 1, 2), (3, 4)):
        pps = {}
        for kc in kcg:
            pps[kc] = (psum.tile([128, 512], F32, tag="bank", name=f"vpr{kc}"),
                       psum.tile([128, 512], F32, tag="bank", name=f"vpi{kc}"))
        for cj in range(4):
            wv_r = ws.tile([128, 256], F32R, tag="w_r")
            wv_i = ws.tile([128, 256], F32R, tag="w_i")
            wv_n = ws.tile([128, 256], F32R, tag="w_n")
            nc.scalar.dma_start(wv_r[:], wv_d[0, 128 * cj:128 * (cj + 1), :])
            nc.gpsimd.dma_start(wv_i[:], wv_d[1, 128 * cj:128 * (cj + 1), :])
            nc.sync.dma_start(wv_n[:], wv_d[2, 128 * cj:128 * (cj + 1), :])
            st = cj == 0
            for kc in kcg:
                k0, szk = K_CHUNKS[kc]
                pr, pi = pps[kc]
                nc.tensor.matmul(pr[:szk, :256], _r(xnTr[:, cj, k0:k0 + szk]),
                                 _r(wv_r[:]), start=st, stop=False)
                nc.tensor.matmul(pr[:szk, :256], _r(xnTi[:, cj, k0:k0 + szk]),
                                 _r(wv_n[:]), start=False, stop=False)
                nc.tensor.matmul(pi[:szk, :256], _r(xnTr[:, cj, k0:k0 + szk]),
                                 _r(wv_i[:]), start=st, stop=False)
                nc.tensor.matmul(pi[:szk, :256], _r(xnTi[:, cj, k0:k0 + szk]),
                                 _r(wv_r[:]), start=False, stop=False)
        for kc in kcg:
            k0, szk = K_CHUNKS[kc]
            pr, pi = pps[kc]
            nc.tensor.matmul(pr[:szk, :256], _r(ones_row[:, :szk]), _r(bv_re[:]),
                             start=False, stop=True)
            nc.tensor.matmul(pi[:szk, :256], _r(ones_row[:, :szk]), _r(bv_im[:]),
                             start=False, stop=True)
            vr_v = pr[:szk, :256].rearrange("p (h d) -> p h d", h=4)
            vi_v = pi[:szk, :256].rearrange("p (h d) -> p h d", h=4)
            nc.vector.tensor_copy(vpk[:szk, kc, :, 0:64], vr_v)
            nc.vector.tensor_copy(vpk[:szk, kc, :, 64:128], vi_v)
    # kc4 (szk=64) packs head pairs along partitions in phase C: odd heads
    # read their v from rows 64:128
    nc.sync.dma_start(vpk[64:128, 4, 1, :], vpk[0:64, 4, 1, :])
    nc.gpsimd.dma_start(vpk[64:128, 4, 3, :], vpk[0:64, 4, 3, :])

    if DEBUG:
        for cj in range(4):
            nc.sync.dma_start(dbg["xnT"][0, 128 * cj:128 * (cj + 1), :],
                              xnTr[:, cj, :].bitcast(F32))
            nc.sync.dma_start(dbg["xnT"][1, 128 * cj:128 * (cj + 1), :],
                              xnTi[:, cj, :].bitcast(F32))
        nc.sync.dma_start(dbg["kA"][:], kA.bitcast(F32))
        nc.sync.dma_start(dbg["kB"][:], kB.bitcast(F32))

    bctx.close()

    # =====================================================================
    # Phase C: per q-chunk: q-proj -> scores -> softmax -> attn@v -> proj
    # =====================================================================
    cctx = contextlib.ExitStack()
    sm = cctx.enter_context(tc.tile_pool(name="sm", bufs=2))
    psC = cctx.enter_context(tc.tile_pool(name="psC", bufs=4, space="PSUM"))
    psOp = cctx.enter_context(tc.tile_pool(name="psOp", bufs=2, space="PSUM"))
    psDn = cctx.enter_context(tc.tile_pool(name="psDn", bufs=2, space="PSUM"))

    def emit_front(q0, nq, hp, qcs):
        """Scores + softmax numerators for head-pair hp; returns ebufs.

        kc 0-3: [szk, 2, nq] tiles (head i in dim 1).  kc 4 (szk=64):
        both heads packed in the partition dim (i0 rows 0:64, i1 64:128)
        so every elementwise op runs one [128, nq] pass instead of two.
        """
        sts = []
        for kc in range(4):
            k0, szk = K_CHUNKS[kc]
            s_t = sm.tile([128, 2, 512], F32, tag="s_t", bufs=4,
                          name=f"s{kc}")
            for i in range(2):
                h = 2 * hp + i
                qc = qcs[h]
                sre = psC.tile([128, 512], F32, tag="bank", name="sre")
                sim = psC.tile([128, 512], F32, tag="bank", name="sim")
                nc.tensor.matmul(sre[:szk, :nq], kA[:, h, k0:k0 + szk],
                                 qc[:, :nq], start=True, stop=True)
                nc.tensor.matmul(sim[:szk, :nq], kB[:, h, k0:k0 + szk],
                                 qc[:, :nq], start=True, stop=True)
                # s = sre^2 + sim^2 in f32
                nc.scalar.activation(s_t[:szk, i, :nq], sre[:szk, :nq],
                                     AF.Square)
                tmp = sm.tile([128, 512], F32, tag="tmp", bufs=2)
                if SQMODE == 0:
                    tmc = sm.tile([128, 512], F32, tag="tmc", bufs=2)
                    nc.vector.tensor_copy(tmc[:szk, :nq], sim[:szk, :nq])
                    nc.vector.tensor_mul(tmp[:szk, :nq], tmc[:szk, :nq],
                                         tmc[:szk, :nq])
                    nc.gpsimd.tensor_add(s_t[:szk, i, :nq], s_t[:szk, i, :nq],
                                         tmp[:szk, :nq])
                else:
                    nc.scalar.activation(tmp[:szk, :nq], sim[:szk, :nq],
                                         AF.Square)
                    nc.vector.tensor_add(s_t[:szk, i, :nq], s_t[:szk, i, :nq],
                                         tmp[:szk, :nq])
            if DEBUG and q0 == 0 and hp == 0 and kc < 2:
                nc.sync.dma_start(dbg["s"][kc, :szk, :, :nq], s_t[:szk, :, :nq])
            sts.append(s_t)
        # kc 4: head-pair packed [128, nq]
        s4 = sm.tile([128, 512], F32, tag="s4", bufs=3)
        sre4 = psC.tile([128, 512], F32, tag="bank", name="sre4")
        sim4 = psC.tile([128, 512], F32, tag="bank", name="sim4")
        for i in range(2):
            h = 2 * hp + i
            # i=0: cols [576:704) -> kc4 data lands in out rows 0:64;
            # i=1: cols [512:640) -> rows 64:128; zero cols pad the rest.
            c0 = 576 - 64 * i
            nc.tensor.matmul(sre4[:, :nq], kA[:, h, c0:c0 + 128],
                             qcs[h][:, :nq], start=i == 0, stop=i == 1)
            nc.tensor.matmul(sim4[:, :nq], kB[:, h, c0:c0 + 128],
                             qcs[h][:, :nq], start=i == 0, stop=i == 1)
        nc.scalar.activation(s4[:, :nq], sre4[:, :nq], AF.Square)
        tmp4 = sm.tile([128, 512], F32, tag="tmp", bufs=2)
        if SQMODE == 0:
            tmc4 = sm.tile([128, 512], F32, tag="tmc", bufs=2)
            nc.vector.tensor_copy(tmc4[:, :nq], sim4[:, :nq])
            nc.vector.tensor_mul(tmp4[:, :nq], tmc4[:, :nq], tmc4[:, :nq])
            nc.gpsimd.tensor_add(s4[:, :nq], s4[:, :nq], tmp4[:, :nq])
        else:
            nc.scalar.activation(tmp4[:, :nq], sim4[:, :nq], AF.Square)
            nc.vector.tensor_add(s4[:, :nq], s4[:, :nq], tmp4[:, :nq])
        # batched LUT runs: |a| = sqrt(s) (or exp(0.5 ln s)), then
        # ebuf = exp(|a| - 50) in bf16.  Softmax is shift-invariant so the
        # constant -50 cancels; it just keeps exp sums in range.
        if SQRTMODE == 0:
            for kc in range(4):
                szk = K_CHUNKS[kc][1]
                nc.scalar.activation(sts[kc][:szk, :, :nq],
                                     sts[kc][:szk, :, :nq], AF.Ln)
            nc.scalar.activation(s4[:, :nq], s4[:, :nq], AF.Ln)
            for kc in range(4):
                szk = K_CHUNKS[kc][1]
                nc.scalar.activation(sts[kc][:szk, :, :nq],
                                     sts[kc][:szk, :, :nq], AF.Exp, scale=0.5)
            nc.scalar.activation(s4[:, :nq], s4[:, :nq], AF.Exp, scale=0.5)
        else:
            for kc in range(4):
                szk = K_CHUNKS[kc][1]
                nc.scalar.activation(sts[kc][:szk, :, :nq],
                                     sts[kc][:szk, :, :nq], AF.Sqrt)
            nc.scalar.activation(s4[:, :nq], s4[:, :nq], AF.Sqrt)
        ebufs = []
        for kc in range(4):
            szk = K_CHUNKS[kc][1]
            ebuf = sm.tile([128, 2, 512], BF16, tag="ebuf", bufs=8,
                           name=f"eb{kc}")
            nc.scalar.activation(ebuf[:szk, :, :nq], sts[kc][:szk, :, :nq],
                                 AF.Exp, bias=nbias[:szk])
            ebufs.append(ebuf)
        eb4 = sm.tile([128, 512], BF16, tag="eb4", bufs=3)
        nc.scalar.activation(eb4[:, :nq], s4[:, :nq], AF.Exp, bias=nbias[:])
        ebufs.append(eb4)
        return ebufs

    def emit_back(q0, nq, hp, ebufs):
        """attn@v + denominators + normalize for head-pair hp."""
        ops = [psOp.tile([128, 512], F32, tag="op", name=f"op{i}")
               for i in range(2)]
        dnp = psDn.tile([128, 512], F32, tag="dn", name="dnp")
        for kc in range(4):
            k0, szk = K_CHUNKS[kc]
            ebuf = ebufs[kc]
            for i in range(2):
                h = 2 * hp + i
                nc.tensor.matmul(ops[i][:, :nq], vpk[:szk, kc, h, :],
                                 ebuf[:szk, i, :nq], start=kc == 0, stop=False)
                nc.tensor.matmul(dnp[32 * i:32 * i + 1, :nq], ones_col[:szk, :],
                                 ebuf[:szk, i, :nq], start=kc == 0, stop=False,
                                 tile_position=(0, 32 * i))
        eb4 = ebufs[4]
        for i in range(2):
            h = 2 * hp + i
            rs = slice(64 * i, 64 * (i + 1))
            mask = ones_h0 if i == 0 else ones_h1
            nc.tensor.matmul(ops[i][:, :nq], vpk[rs, 4, h, :],
                             eb4[rs, :nq], start=False, stop=True)
            nc.tensor.matmul(dnp[32 * i:32 * i + 1, :nq], mask[:, :],
                             eb4[:, :nq], start=False, stop=True,
                             tile_position=(0, 32 * i))
        dnt = sm.tile([1, 1024], F32, tag="dnt", bufs=2)
        dns = sm.tile([1, 1024], F32, tag="dns", bufs=2)
        dnr = sm.tile([1, 1024], F32R, tag="dnr", bufs=2)
        for i in range(2):
            c = slice(512 * i, 512 * i + nq)
            # ACT partition-shifts row 32i of PSUM into partition 0;
            # reciprocal_approx_fast is broken for base_partition != 0 on HW
            nc.scalar.copy(dnt[0:1, c], dnp[32 * i:32 * i + 1, :nq])
            if RECIPMODE == 0:
                nc.vector.reciprocal_approx_fast(dns[0:1, c], dnt[0:1, c])
                nc.vector.tensor_copy(dnr[0:1, c], dns[0:1, c])
            else:
                nc.scalar.activation(dns[0:1, c], dnt[0:1, c], AF.Ln)
                nc.scalar.activation(dnr[0:1, c], dns[0:1, c], AF.Exp,
                                     scale=-1.0)
        oris = []
        for i in range(2):
            osb = sm.tile([128, 512], F32, tag="osb", bufs=2, name=f"osb{i}")
            nc.scalar.copy(osb[:, :nq], ops[i][:, :nq])
            rbp = psC.tile([128, 512], F32, tag="bank", name="rbp")
            nc.tensor.matmul(rbp[:, :nq], _r(ones128[0:1, :]),
                             dnr[0:1, 512 * i:512 * i + nq],
                             start=True, stop=True)
            ori = sm.tile([128, 512], F16, tag="ori", bufs=5, name=f"ori{i}")
            nc.vector.tensor_mul(ori[:, :nq], osb[:, :nq], rbp[:, :nq])
            oris.append(ori)
        return oris

    def emit_proj(q0, nq, oris):
        """Partial output projection for one chunk; oris = [h0..h3]."""
        for cc in range(4):
            cs = slice(128 * cc, 128 * (cc + 1))
            pr = psC.tile([128, 512], F32, tag="bank", name="pjr")
            pi = psC.tile([128, 512], F32, tag="bank", name="pji")
            for h in range(4):
                st = h == 0
                sp = h == 3
                nc.tensor.matmul(pr[:, :nq], wps[:, 0, h, cs], oris[h][:, :nq],
                                 start=st, stop=sp)
                nc.tensor.matmul(pi[:, :nq], wps[:, 1, h, cs], oris[h][:, :nq],
                                 start=st, stop=sp)
            o1 = sm.tile([128, 512], F16, tag="o1", bufs=2)
            o2 = sm.tile([128, 512], F16, tag="o2", bufs=2)
            nc.vector.tensor_copy(o1[:, :nq], pr[:, :nq])
            nc.vector.tensor_copy(o2[:, :nq], pi[:, :nq])
            nc.gpsimd.dma_start(outT_d[0, cs, q0:q0 + nq], o1[:, :nq])
            nc.sync.dma_start(outT_d[1, cs, q0:q0 + nq], o2[:, :nq])

    # software pipeline, two half-steps deep: back(H[i-2]) is emitted after
    # front(H[i]) so its ebufs have had two full half-steps of ACT/DVE time;
    # the PE meanwhile streams sc(H[i]) + av(H[i-2]) + qp/proj with no stall.
    halves = [(q0, nq, hp) for (q0, nq) in Q_CHUNKS for hp in range(2)]
    qcs_by_q0 = dict(qcs_early)
    fronts = []
    ori_by_q0 = {}

    def _drain(idx):
        bq0, bnq, bhp, bebufs = fronts[idx]
        oris = emit_back(bq0, bnq, bhp, bebufs)
        ori_by_q0.setdefault(bq0, []).extend(oris)
        if bhp == 1:
            emit_proj(bq0, bnq, ori_by_q0.pop(bq0))

    for i, (q0, nq, hp) in enumerate(halves):
        ebufs = emit_front(q0, nq, hp, qcs_by_q0[q0])
        fronts.append((q0, nq, hp, ebufs))
        qi = i // 2
        if hp == 1 and qi + 2 < len(Q_CHUNKS):
            nq0, nnq = Q_CHUNKS[qi + 2]
            qcs_by_q0[nq0] = emit_qp(nq0, nnq, psC)
        if i >= 2:
            _drain(i - 2)
    _drain(len(halves) - 2)
    _drain(len(halves) - 1)

    cctx.close()
    ctx.close()


# =========================================================================
# Host side
# =========================================================================

def _f32(x):
    return np.ascontiguousarray(x, dtype=np.float32)


def _f16(x):
    return np.ascontiguousarray(np.asarray(x, dtype=np.float16))


def host_prep(x_re, x_im, Wq, Wkv, Wproj, bproj, sr_w, sr_b, gain, bias):
    x_re = np.asarray(x_re)
    x_im = np.asarray(x_im)
    Wq = np.asarray(Wq)
    Wkv = np.asarray(Wkv)
    Wproj = np.asarray(Wproj)
    sr_w = np.asarray(sr_w)
    sr_b = np.asarray(sr_b)
    gain = np.asarray(gain)
    bias = np.asarray(bias)

    Wkv_eff = gain[:, None] * Wkv
    bkv_full = bias @ Wkv
    Wc = sr_w.transpose(2, 3, 1, 0).reshape(4 * C, C)

    def planes3f(w):
        return np.stack([_f32(w.real), _f32(w.imag), _f32(-w.imag)])

    in_maps = []
    for core in range(8):
        b, g = core // 2, core % 2
        cols = slice(256 * g, 256 * (g + 1))
        wk_c = Wkv_eff[:, :C][:, cols] * SCALE
        wv_c = Wkv_eff[:, C:][:, cols]
        bk_c = bkv_full[:C][cols] * SCALE
        bv_c = bkv_full[C:][cols]
        xs_c = np.stack([x_re[b].T, x_im[b].T])  # [2, C, N]
        xsp = xs_c.reshape(2, C, HR, 2, HR, 2)
        xP = np.stack([xsp[:, :, :, p, :, q].reshape(2, C, NK)
                       for p in range(2) for q in range(2)], axis=1)
        # wpA_h = [Wp_re rows of head h ; -Wp_im rows], wpB_h = [im ; re]
        wp_blk = Wproj[256 * g:256 * (g + 1), :]  # [256, C] complex
        wpab = np.zeros((2, 4, 128, C), np.float16)
        for h in range(4):
            rows = wp_blk[64 * h:64 * (h + 1), :]
            wpab[0, h, 0:64] = _f16(rows.real)
            wpab[0, h, 64:128] = _f16(-rows.imag)
            wpab[1, h, 0:64] = _f16(rows.imag)
            wpab[1, h, 64:128] = _f16(rows.real)
        m = {
            "xT": _f32(xs_c),
            "xP": _f32(xP.reshape(2, 4 * C, NK)),
            "wc": planes3f(Wc),
            "srb": np.stack([_f32(sr_b.real), _f32(sr_b.imag)]),
            "ones": np.ones((1, 512), np.float32),
            "wq": planes3f(Wq[:, cols]),
            "wk": planes3f(wk_c),
            "wv": planes3f(wv_c),
            "wpab": wpab,
            "bkv": np.stack([
                np.stack([_f32(bk_c.real), _f32(bv_c.real)]),
                np.stack([_f32(bk_c.imag), _f32(bv_c.imag)]),
            ]),
        }
        in_maps.append(m)
    return in_maps


_NC_CACHE = None


def _get_nc():
    global _NC_CACHE
    if _NC_CACHE is None:
        _NC_CACHE = build_nc()
    return _NC_CACHE


def kernel(x_re, x_im, Wq, Wkv, Wproj, bproj, sr_w, sr_b, gain, bias, H, W):
    from concourse.bass_utils import run_bass_kernel_spmd

    nc = _get_nc()
    in_maps = host_prep(x_re, x_im, Wq, Wkv, Wproj, bproj, sr_w, sr_b, gain, bias)
    res = run_bass_kernel_spmd(nc, in_maps, list(range(8)))
    bproj = np.asarray(bproj)
    out = np.zeros((B, N, C), dtype=np.complex64)
    for b in range(B):
        p0 = res.results[2 * b]["outT"].astype(np.float32)
        p1 = res.results[2 * b + 1]["outT"].astype(np.float32)
        acc = (p0[0] + p1[0]).T + 1j * (p0[1] + p1[1]).T
        out[b] = acc + bproj[None, :]
    return out


# revision 26
# speedup vs baseline: 1.2652x; 1.0540x over previous
"""Trainium2 Bass kernel for complex-valued spatial-reduction attention.

x: [B=4, N=2304, C=512] complex64 (re/im f32 planes), H=W=48, 8 heads,
head_dim 64, sr_ratio 2 -> Nk=576.

Sharding: 8 cores = 4 batches x 2 head-groups (4 heads each). Each core:
sr-conv over full C, complex LayerNorm, q/k/v for its heads,
softmax(|q.k^T|) attention, attn @ v, partial output projection.
Host sums the two partials per batch and adds bproj.

v2 structure: scores use K=128 packing (kA=[kr;-ki], kB=[ki;kr],
qcat=[qr;qi]); q-projection and output projection run inside the
attention loop (q/attn-out never round-trip DRAM); softmax runs
Square(ACT f32) + sim^2/add (DVE f32) + batched Ln/Exp runs;
denominator reciprocal on DVE (no act-table thrash); v/attn/proj in f16.
"""

import os
import contextlib

import numpy as np
import ml_dtypes

import concourse.bass as bass
import concourse.mybir as mybir
import concourse.tile as tile
from concourse import bacc
from concourse.masks import make_identity

BF16 = mybir.dt.bfloat16
F16 = mybir.dt.float16
F32 = mybir.dt.float32
F32R = mybir.dt.float32r
AF = mybir.ActivationFunctionType
ALU = mybir.AluOpType

B, N, C, HEADS, HD, SR = 4, 2304, 512, 8, 64, 2
NK = 576
HR = 24
EPS = 1e-5
SCALE = HD ** -0.5  # folded into Wk host-side

# each core computes 288 of the 576 conv rows; pairs exchange via AllGather
M_GROUPS = [
    [(0, 5), (5, 5), (10, 2)],
]
K_CHUNKS = [(0, 128), (128, 128), (256, 128), (384, 128), (512, 64)]
Q_CHUNKS = [(0, 512), (512, 512), (1024, 512), (1536, 512), (2048, 256)]

# 0 = ln/exp sqrt (safe), 1 = ACT Sqrt LUT (faster; precision probe)
SQRTMODE = int(os.environ.get("KBUILD_SQRT", "1"))
# 0 = DVE tensor_mul(sim, sim) reading same PSUM AP twice; 1 = ACT Square
SQMODE = int(os.environ.get("KBUILD_SQ", "0"))
DEBUG = bool(int(os.environ.get("KBUILD_DEBUG", "0")))
# 1 = baseline ln/exp reciprocal instead of reciprocal_approx_fast
RECIPMODE = int(os.environ.get("KBUILD_RECIP", "0"))


def _r(ap):
    return ap.bitcast(F32R)


def build_nc():
    nc = bacc.Bacc("TRN2", target_bir_lowering=False, debug=False, num_devices=8)

    xT_d = nc.dram_tensor("xT", [2, C, N], F32R, kind="ExternalInput")
    xP_d = nc.dram_tensor("xP", [2, 4 * C, 288], F32R, kind="ExternalInput")
    wc_d = nc.dram_tensor("wc", [3, 4 * C, C], F32R, kind="ExternalInput")
    srb_d = nc.dram_tensor("srb", [2, C], F32R, kind="ExternalInput")
    ones_d = nc.dram_tensor("ones", [1, 512], F32R, kind="ExternalInput")
    wq_d = nc.dram_tensor("wq", [3, C, 256], F32R, kind="ExternalInput")
    wk_d = nc.dram_tensor("wk", [3, C, 256], F32R, kind="ExternalInput")
    wv_d = nc.dram_tensor("wv", [3, C, 256], F32R, kind="ExternalInput")
    wpab_d = nc.dram_tensor("wpab", [2, 4, 128, C], F16, kind="ExternalInput")
    bkv_d = nc.dram_tensor("bkv", [2, 2, 256], F32R, kind="ExternalInput")
    outT_d = nc.dram_tensor("outT", [2, C, N], F16, kind="ExternalOutput")
    ag_in_d = nc.dram_tensor("ag_in", [2, C, 288], F32R)
    ag_out_d = nc.dram_tensor("ag_out", [2, 2, C, 288], F32R)
    dbg = {}
    if DEBUG:
        dbg["kA"] = nc.dram_tensor("dbg_kA", [128, 4, NK], F32, kind="ExternalOutput")
        dbg["kB"] = nc.dram_tensor("dbg_kB", [128, 4, NK], F32, kind="ExternalOutput")
        dbg["qc"] = nc.dram_tensor("dbg_qc", [4, 128, 512], F32, kind="ExternalOutput")
        dbg["s"] = nc.dram_tensor("dbg_s", [2, 128, 2, 512], F32, kind="ExternalOutput")
        dbg["xnT"] = nc.dram_tensor("dbg_xnT", [2, C, NK], F32, kind="ExternalOutput")

    with tile.TileContext(nc) as tc:
        _body(nc, tc, xT_d, xP_d, wc_d, srb_d, ones_d, wq_d, wk_d, wv_d,
              wpab_d, bkv_d, outT_d, ag_in_d, ag_out_d, dbg)

    nc.compile()
    return nc


def _ln_chunk(nc, work, stats, cre, cim, sz):
    """Complex LayerNorm for one [sz, C] chunk in PSUM -> (xnr, xni)."""
    inv_c = 1.0 / C
    re_sb = work.tile([128, C], F32, tag="ln_re")
    im_sb = work.tile([128, C], F32, tag="ln_im")
    sum_r = stats.tile([128, 1], F32, tag="sum_r")
    sum_i = stats.tile([128, 1], F32, tag="sum_i")
    nc.vector.tensor_copy(re_sb[:sz], cre[:sz, :])
    nc.vector.tensor_copy(im_sb[:sz], cim[:sz, :])
    nc.vector.tensor_reduce(sum_r[:sz], re_sb[:sz], mybir.AxisListType.X, ALU.add)
    nc.vector.tensor_reduce(sum_i[:sz], im_sb[:sz], mybir.AxisListType.X, ALU.add)
    junk = work.tile([128, C], F32, tag="ln_junk", bufs=1)
    sxx = stats.tile([128, 1], F32, tag="sxx")
    sii = stats.tile([128, 1], F32, tag="sii")
    sxi = stats.tile([128, 1], F32, tag="sxi")
    nc.vector.tensor_mul(junk[:sz], re_sb[:sz], re_sb[:sz])
    nc.vector.tensor_reduce(sxx[:sz], junk[:sz], mybir.AxisListType.X, ALU.add)
    nc.vector.tensor_mul(junk[:sz], im_sb[:sz], im_sb[:sz])
    nc.vector.tensor_reduce(sii[:sz], junk[:sz], mybir.AxisListType.X, ALU.add)
    nc.vector.tensor_mul(junk[:sz], re_sb[:sz], im_sb[:sz])
    nc.vector.tensor_reduce(sxi[:sz], junk[:sz], mybir.AxisListType.X, ALU.add)
    mr = stats.tile([128, 1], F32, tag="mr")
    mi = stats.tile([128, 1], F32, tag="mi")
    nc.vector.tensor_scalar_mul(mr[:sz], sum_r[:sz], inv_c)
    nc.vector.tensor_scalar_mul(mi[:sz], sum_i[:sz], inv_c)
    vre = stats.tile([128, 1], F32, tag="vre")
    vim = stats.tile([128, 1], F32, tag="vim")
    tA = stats.tile([128, 1], F32, tag="tA")
    tB = stats.tile([128, 1], F32, tag="tB")
    nc.vector.tensor_sub(tA[:sz], sxx[:sz], sii[:sz])
    nc.vector.tensor_scalar_mul(tA[:sz], tA[:sz], inv_c)
    nc.vector.tensor_mul(vre[:sz], mr[:sz], mr[:sz])
    nc.vector.tensor_mul(tB[:sz], mi[:sz], mi[:sz])
    nc.vector.tensor_sub(vre[:sz], vre[:sz], tB[:sz])
    nc.vector.tensor_sub(vre[:sz], tA[:sz], vre[:sz])
    nc.vector.tensor_scalar_add(vre[:sz], vre[:sz], EPS)
    nc.vector.tensor_mul(tB[:sz], mr[:sz], mi[:sz])
    nc.vector.tensor_scalar_mul(tB[:sz], tB[:sz], 2.0)
    nc.vector.tensor_scalar_mul(vim[:sz], sxi[:sz], 2.0 * inv_c)
    nc.vector.tensor_sub(vim[:sz], vim[:sz], tB[:sz])
    r2 = stats.tile([128, 1], F32, tag="r2")
    nc.vector.tensor_mul(r2[:sz], vre[:sz], vre[:sz])
    nc.vector.tensor_mul(tB[:sz], vim[:sz], vim[:sz])
    nc.vector.tensor_add(r2[:sz], r2[:sz], tB[:sz])

    def _sqrt_newton(out, x, sc):
        # y0 = LUT sqrt(sc*x); y1 = 0.5*(y0 + sc*x/y0)  (one Newton step)
        y0 = stats.tile([128, 1], F32, tag="nw_y0")
        nc.scalar.activation(y0[:sz], x[:sz], AF.Sqrt, scale=sc)
        yr = stats.tile([128, 1], F32, tag="nw_yr")
        nc.vector.tensor_scalar_add(y0[:sz], y0[:sz], 1e-30)
        nc.vector.reciprocal(yr[:sz], y0[:sz])
        nc.vector.tensor_mul(yr[:sz], yr[:sz], x[:sz])
        if sc != 1.0:
            nc.vector.tensor_scalar_mul(yr[:sz], yr[:sz], sc)
        nc.vector.tensor_add(out[:sz], y0[:sz], yr[:sz])
        nc.vector.tensor_scalar_mul(out[:sz], out[:sz], 0.5)

    rr = stats.tile([128, 1], F32, tag="rr")
    _sqrt_newton(rr, r2, 1.0)
    srt = stats.tile([128, 1], F32, tag="srt")
    sia = stats.tile([128, 1], F32, tag="sia")
    nc.vector.tensor_add(tA[:sz], rr[:sz], vre[:sz])
    _sqrt_newton(srt, tA, 0.5)
    nc.vector.tensor_sub(tA[:sz], rr[:sz], vre[:sz])
    _sqrt_newton(sia, tA, 0.5)
    sgn = stats.tile([128, 1], F32, tag="sgn")
    nc.scalar.activation(sgn[:sz], vim[:sz], AF.Sign)
    nc.vector.tensor_mul(sia[:sz], sia[:sz], sgn[:sz])
    rin = stats.tile([128, 1], F32, tag="rin")
    nc.vector.reciprocal(rin[:sz], rr[:sz])
    wr = stats.tile([128, 1], F32, tag="wr")
    wn = stats.tile([128, 1], F32, tag="wn")  # = -w_im
    nc.vector.tensor_mul(wr[:sz], srt[:sz], rin[:sz])
    nc.vector.tensor_mul(wn[:sz], sia[:sz], rin[:sz])
    aT = work.tile([128, C], F32, tag="ln_a")
    bT = work.tile([128, C], F32, tag="ln_b")
    xnr = work.tile([128, C], F32, tag="ln_xnr")
    xni = work.tile([128, C], F32, tag="ln_xni")
    nc.vector.tensor_scalar(aT[:sz], re_sb[:sz], mr[:sz], wr[:sz],
                            ALU.subtract, ALU.mult)
    nc.vector.tensor_scalar(bT[:sz], im_sb[:sz], mi[:sz], wn[:sz],
                            ALU.subtract, ALU.mult)
    nc.vector.tensor_add(xnr[:sz], aT[:sz], bT[:sz])
    nc.vector.tensor_scalar(aT[:sz], re_sb[:sz], mr[:sz], wn[:sz],
                            ALU.subtract, ALU.mult)
    nc.vector.tensor_scalar(bT[:sz], im_sb[:sz], mi[:sz], wr[:sz],
                            ALU.subtract, ALU.mult)
    nc.vector.tensor_sub(xni[:sz], bT[:sz], aT[:sz])
    return xnr, xni


def _body(nc, tc, xT_d, xP_d, wc_d, srb_d, ones_d, wq_d, wk_d, wv_d,
          wpab_d, bkv_d, outT_d, ag_in_d, ag_out_d, dbg=None):
    ctx = contextlib.ExitStack()
    consts = ctx.enter_context(tc.tile_pool(name="consts", bufs=1))
    big = ctx.enter_context(tc.tile_pool(name="big", bufs=1))
    xqp = ctx.enter_context(tc.tile_pool(name="xqp", bufs=4))
    qd = ctx.enter_context(tc.tile_pool(name="qd", bufs=4))
    qcp = ctx.enter_context(tc.tile_pool(name="qcp", bufs=8))

    # ---- constants ----
    ident = consts.tile([128, 128], F32, tag="ident")
    make_identity(nc, ident)
    ones_col = consts.tile([128, 1], BF16, tag="ones_col")
    nc.vector.memset(ones_col, 1.0)
    ones_h0 = consts.tile([128, 1], BF16, tag="ones_h0")
    nc.vector.memset(ones_h0, 0.0)
    nc.vector.memset(ones_h0[0:64], 1.0)
    ones_h1 = consts.tile([128, 1], BF16, tag="ones_h1")
    nc.vector.memset(ones_h1, 0.0)
    nc.vector.memset(ones_h1[64:128], 1.0)
    ones128 = consts.tile([128, 128], F32, tag="ones128")
    nc.vector.memset(ones128, 1.0)
    ones_row = consts.tile([1, 512], F32R, tag="ones_row")
    nc.sync.dma_start(ones_row[:], ones_d[:])
    nbias = consts.tile([128, 1], F32, tag="nbias")
    nc.vector.memset(nbias, -50.0)

    srb_re = consts.tile([1, C], F32R, tag="srb_re")
    srb_im = consts.tile([1, C], F32R, tag="srb_im")
    nc.sync.dma_start(srb_re[:], srb_d[0:1, :])
    nc.sync.dma_start(srb_im[:], srb_d[1:2, :])
    bk_re = consts.tile([1, 256], F32R, tag="bk_re")
    bk_im = consts.tile([1, 256], F32R, tag="bk_im")
    bv_re = consts.tile([1, 256], F32R, tag="bv_re")
    bv_im = consts.tile([1, 256], F32R, tag="bv_im")
    nc.sync.dma_start(bk_re[:], bkv_d[0:1, 0, :])
    nc.sync.dma_start(bv_re[:], bkv_d[0:1, 1, :])
    nc.sync.dma_start(bk_im[:], bkv_d[1:2, 0, :])
    nc.sync.dma_start(bv_im[:], bkv_d[1:2, 1, :])

    # ---- persistent SBUF ----
    xnTr = big.tile([128, 4, NK], F32R, tag="xnTr")
    xnTi = big.tile([128, 4, NK], F32R, tag="xnTi")
    # kA_h = [kr_h ; -ki_h], kB_h = [ki_h ; kr_h]  (partition dim 64+64)
    kA = big.tile([128, 4, 704], F32R, tag="kA")
    kB = big.tile([128, 4, 704], F32R, tag="kB")
    nc.vector.memset(kA.bitcast(F32)[:, :, 512:576], 0.0)
    nc.vector.memset(kA.bitcast(F32)[:, :, 640:704], 0.0)
    nc.vector.memset(kB.bitcast(F32)[:, :, 512:576], 0.0)
    nc.vector.memset(kB.bitcast(F32)[:, :, 640:704], 0.0)
    vpk = big.tile([128, 5, 4, 128], BF16, tag="vpk")
    wqs = big.tile([128, 3, 4, 256], F32R, tag="wqs")
    nc.sync.dma_start(wqs[:], wq_d.rearrange("s (j p) n -> p s j n", p=128))
    wps = big.tile([128, 2, 4, C], F16, tag="wps")
    nc.sync.dma_start(wps[:], wpab_d.rearrange("a h p n -> p a h n"))

    xT_v = [xT_d[pl].rearrange("(j p) t -> p j t", p=128) for pl in (0, 1)]

    # =====================================================================
    # Phase B: conv (f32r) + LayerNorm + transposes + k/v projections
    # =====================================================================
    bctx = contextlib.ExitStack()
    xs = bctx.enter_context(tc.tile_pool(name="xs", bufs=4))
    wcp = bctx.enter_context(tc.tile_pool(name="wcp", bufs=2))
    ws = bctx.enter_context(tc.tile_pool(name="ws", bufs=2))
    work = bctx.enter_context(tc.tile_pool(name="work", bufs=2))
    stats = bctx.enter_context(tc.tile_pool(name="stats", bufs=2))
    psum = bctx.enter_context(tc.tile_pool(name="psumB", bufs=7, space="PSUM"))

    def emit_qp(q0, nq, pspool):
        """q-projection for one chunk; returns qc[h] = [qr_h;qi_h] tiles."""
        prs = []
        for half in range(2):
            prs.append((pspool.tile([128, 512], F32, tag="bank", name=f"qpr{half}"),
                        pspool.tile([128, 512], F32, tag="bank", name=f"qpi{half}")))
        for cj in range(4):
            xq_r = xqp.tile([128, 512], F32R, tag="xq_r")
            xq_i = xqp.tile([128, 512], F32R, tag="xq_i")
            nc.gpsimd.dma_start(xq_r[:, :nq], xT_v[0][:, cj, q0:q0 + nq])
            nc.sync.dma_start(xq_i[:, :nq], xT_v[1][:, cj, q0:q0 + nq])
            st = cj == 0
            sp = cj == 3
            for half in range(2):
                hs = slice(128 * half, 128 * (half + 1))
                pr, pi = prs[half]
                nc.tensor.matmul(pr[:, :nq], wqs[:, 0, cj, hs], _r(xq_r[:, :nq]),
                                 start=st, stop=False)
                nc.tensor.matmul(pr[:, :nq], wqs[:, 2, cj, hs], _r(xq_i[:, :nq]),
                                 start=False, stop=sp)
                nc.tensor.matmul(pi[:, :nq], wqs[:, 1, cj, hs], _r(xq_r[:, :nq]),
                                 start=st, stop=False)
                nc.tensor.matmul(pi[:, :nq], wqs[:, 0, cj, hs], _r(xq_i[:, :nq]),
                                 start=False, stop=sp)
        qcs = []
        for half in range(2):
            pr, pi = prs[half]
            qr_sb = qd.tile([128, 512], F32R, tag="qd", name=f"qr{half}")
            qi_sb = qd.tile([128, 512], F32R, tag="qd", name=f"qi{half}")
            nc.scalar.copy(qr_sb[:, :nq], pr[:, :nq])
            nc.scalar.copy(qi_sb[:, :nq], pi[:, :nq])
            for j in range(2):
                h = 2 * half + j
                qc = qcp.tile([128, 512], F32R, tag="qc", name=f"qc{h}")
                rs = slice(64 * j, 64 * (j + 1))
                nc.sync.dma_start(qc[0:64, :nq], qr_sb[rs, :nq])
                nc.gpsimd.dma_start(qc[64:128, :nq], qi_sb[rs, :nq])
                if DEBUG and q0 == 0:
                    nc.sync.dma_start(dbg["qc"][h, :, :nq], qc[:, :nq].bitcast(F32))
                qcs.append(qc)
        return qcs


    qcs_early = {}

    for mg in M_GROUPS:
        tg0 = mg[0][0] * HR
        tgs = sum(nh for _, nh in mg) * HR
        cps = []
        for (hr0, nh) in mg:
            cre = psum.tile([128, C], F32, tag="bank")
            cim = psum.tile([128, C], F32, tag="bank")
            cps.append((cre, cim, hr0 * HR - tg0, hr0 * HR, nh * HR))

        first = [[True, True] for _ in mg]
        for kk in range(16):
            xp_r = xs.tile([128, 288], F32R, tag="xp_r")
            xp_i = xs.tile([128, 288], F32R, tag="xp_i")
            nc.gpsimd.dma_start(xp_r[:, :tgs], xP_d[0, 128 * kk:128 * (kk + 1),
                                                    tg0:tg0 + tgs])
            nc.scalar.dma_start(xp_i[:, :tgs], xP_d[1, 128 * kk:128 * (kk + 1),
                                                  tg0:tg0 + tgs])
            w_re = wcp.tile([128, C], F32R, tag="wc_re")
            w_im = wcp.tile([128, C], F32R, tag="wc_im")
            w_in = wcp.tile([128, C], F32R, tag="wc_in")
            nc.scalar.dma_start(w_re[:], wc_d[0, 128 * kk:128 * (kk + 1), :])
            nc.sync.dma_start(w_im[:], wc_d[1, 128 * kk:128 * (kk + 1), :])
            nc.gpsimd.dma_start(w_in[:], wc_d[2, 128 * kk:128 * (kk + 1), :])
            for mi_, (cre, cim, off, t0, sz) in enumerate(cps):
                pat_r = xp_r[:, off:off + sz]
                pat_i = xp_i[:, off:off + sz]
                nc.tensor.matmul(cre[:sz, :], _r(pat_r), _r(w_re[:]),
                                 start=first[mi_][0], stop=False)
                nc.tensor.matmul(cim[:sz, :], _r(pat_r), _r(w_im[:]),
                                 start=first[mi_][1], stop=False)
                first[mi_] = [False, False]
                nc.tensor.matmul(cre[:sz, :], _r(pat_i), _r(w_in[:]),
                                 start=False, stop=False)
                nc.tensor.matmul(cim[:sz, :], _r(pat_i), _r(w_re[:]),
                                 start=False, stop=False)
        for (cre, cim, off, t0, sz) in cps:
            nc.tensor.matmul(cre[:sz, :], _r(ones_row[:, :sz]), _r(srb_re[:]),
                             start=False, stop=True)
            nc.tensor.matmul(cim[:sz, :], _r(ones_row[:, :sz]), _r(srb_im[:]),
                             start=False, stop=True)

        # q-projection fills the PE while the LayerNorm chain runs
        qcs_early[Q_CHUNKS[0][0]] = emit_qp(*Q_CHUNKS[0], psum)

        # ---- LayerNorm + transpose into local xnT rows [0:288) ----
        for (cre, cim, off, t0, sz) in cps:
            xnr, xni = _ln_chunk(nc, work, stats, cre, cim, sz)
            for cj in range(4):
                for src, dst in ((xnr, xnTr), (xni, xnTi)):
                    pt = psum.tile([128, 128], F32, tag="bank")
                    nc.tensor.transpose(pt[:, :sz],
                                        src[:sz, 128 * cj:128 * (cj + 1)],
                                        ident[:sz, :sz])
                    nc.vector.tensor_copy(dst[:, cj, t0:t0 + sz], pt[:, :sz])

    # ---- exchange halves with the paired core (AllGather over pairs) ----
    for pl, xn in ((0, xnTr), (1, xnTi)):
        for cj in range(4):
            nc.sync.dma_start(ag_in_d[pl, 128 * cj:128 * (cj + 1), :],
                              xn[:, cj, 0:288])
    nc.gpsimd.collective_compute(
        "AllGather", mybir.AluOpType.bypass,
        ins=[ag_in_d[:]], outs=[ag_out_d[:]],
        replica_groups=[[0, 1], [2, 3], [4, 5], [6, 7]])
    qcs_early[Q_CHUNKS[1][0]] = emit_qp(*Q_CHUNKS[1], psum)
    for pl, xn in ((0, xnTr), (1, xnTi)):
        for cj in range(4):
            for p in range(2):
                eng = nc.sync if (cj + p) % 2 else nc.scalar
                eng.dma_start(xn[:, cj, 288 * p:288 * (p + 1)],
                              ag_out_d[p, pl, 128 * cj:128 * (cj + 1), :])

    # ---- k^T projection -> kA/kB (packed K=128 layout) ----
    # kn holds the negated / positive ki pieces that need partition shifts.
    kn = big.tile([128, 2, 704], F32R, tag="kn")
    nc.vector.memset(kn.bitcast(F32)[:, :, 512:576], 0.0)
    nc.vector.memset(kn.bitcast(F32)[:, :, 640:704], 0.0)
    for half in range(2):
        hs = slice(128 * half, 128 * (half + 1))
        p512r = psum.tile([128, 512], F32, tag="bank")
        p512i = psum.tile([128, 512], F32, tag="bank")
        p64r = psum.tile([128, 512], F32, tag="bank")
        p64i = psum.tile([128, 512], F32, tag="bank")
        for cj in range(4):
            wk_r = ws.tile([128, 256], F32R, tag="w_r")
            wk_i = ws.tile([128, 256], F32R, tag="w_i")
            wk_n = ws.tile([128, 256], F32R, tag="w_n")
            nc.scalar.dma_start(wk_r[:], wk_d[0, 128 * cj:128 * (cj + 1), :])
            nc.gpsimd.dma_start(wk_i[:], wk_d[1, 128 * cj:128 * (cj + 1), :])
            nc.sync.dma_start(wk_n[:], wk_d[2, 128 * cj:128 * (cj + 1), :])
            st = cj == 0
            for (pr, pi, n0, nn) in ((p512r, p512i, 0, 512), (p64r, p64i, 512, 64)):
                nc.tensor.matmul(pr[:, :nn], _r(wk_r[:, hs]),
                                 _r(xnTr[:, cj, n0:n0 + nn]), start=st, stop=False)
                nc.tensor.matmul(pr[:, :nn], _r(wk_n[:, hs]),
                                 _r(xnTi[:, cj, n0:n0 + nn]), start=False, stop=False)
                nc.tensor.matmul(pi[:, :nn], _r(wk_i[:, hs]),
                                 _r(xnTr[:, cj, n0:n0 + nn]), start=st, stop=False)
                nc.tensor.matmul(pi[:, :nn], _r(wk_r[:, hs]),
                                 _r(xnTi[:, cj, n0:n0 + nn]), start=False, stop=False)
        h0, h1 = 2 * half, 2 * half + 1
        for (pr, pi, n0, nn) in ((p512r, p512i, 0, 512), (p64r, p64i, 576, 64)):
            nc.tensor.matmul(pr[:, :nn], _r(bk_re[:, hs]), _r(ones_row[:, :nn]),
                             start=False, stop=True)
            nc.tensor.matmul(pi[:, :nn], _r(bk_im[:, hs]), _r(ones_row[:, :nn]),
                             start=False, stop=True)
            ns = slice(n0, n0 + nn)
            # aligned drains (no partition shift):
            nc.vector.tensor_copy(kA[0:64, h0, ns], pr[0:64, :nn])       # kr_h0
            nc.vector.tensor_copy(kB[64:128, h1, ns], pr[64:128, :nn])   # kr_h1
            nc.vector.tensor_copy(kB[0:64, h0, ns], pi[0:64, :nn])       # ki_h0
            nc.vector.tensor_scalar_mul(kA[64:128, h1, ns], pi[64:128, :nn],
                                        -1.0)                            # -ki_h1
            nc.vector.tensor_scalar_mul(kn[0:64, half, ns], pi[0:64, :nn],
                                        -1.0)                            # -ki_h0
            nc.vector.tensor_copy(kn[64:128, half, ns], pi[64:128, :nn])  # ki_h1
        # partition-shifting copies via SBUF->SBUF DMA
        nc.sync.dma_start(kA[64:128, h0, :], kn[0:64, half, :])     # -ki_h0
        nc.sync.dma_start(kB[0:64, h1, :], kn[64:128, half, :])     # ki_h1
        nc.gpsimd.dma_start(kA[0:64, h1, :], kB[64:128, h1, :])     # kr_h1
        nc.gpsimd.dma_start(kB[64:128, h0, :], kA[0:64, h0, :])     # kr_h0

    # ---- v projection (f32r) -> vpk f16 ----
    for kcg in ((0, 1, 2), (3, 4)):
        pps = {}
        for kc in kcg:
            pps[kc] = (psum.tile([128, 512], F32, tag="bank", name=f"vpr{kc}"),
                       psum.tile([128, 512], F32, tag="bank", name=f"vpi{kc}"))
        for cj in range(4):
            wv_r = ws.tile([128, 256], F32R, tag="w_r")
            wv_i = ws.tile([128, 256], F32R, tag="w_i")
            wv_n = ws.tile([128, 256], F32R, tag="w_n")
            nc.scalar.dma_start(wv_r[:], wv_d[0, 128 * cj:128 * (cj + 1), :])
            nc.gpsimd.dma_start(wv_i[:], wv_d[1, 128 * cj:128 * (cj + 1), :])
            nc.sync.dma_start(wv_n[:], wv_d[2, 128 * cj:128 * (cj + 1), :])
            st = cj == 0
            for kc in kcg:
                k0, szk = K_CHUNKS[kc]
                pr, pi = pps[kc]
                nc.tensor.matmul(pr[:szk, :256], _r(xnTr[:, cj, k0:k0 + szk]),
                                 _r(wv_r[:]), start=st, stop=False)
                nc.tensor.matmul(pr[:szk, :256], _r(xnTi[:, cj, k0:k0 + szk]),
                                 _r(wv_n[:]), start=False, stop=False)
                nc.tensor.matmul(pi[:szk, :256], _r(xnTr[:, cj, k0:k0 + szk]),
                                 _r(wv_i[:]), start=st, stop=False)
                nc.tensor.matmul(pi[:szk, :256], _r(xnTi[:, cj, k0:k0 + szk]),
                                 _r(wv_r[:]), start=False, stop=False)
        for kc in kcg:
            k0, szk = K_CHUNKS[kc]
            pr, pi = pps[kc]
            nc.tensor.matmul(pr[:szk, :256], _r(ones_row[:, :szk]), _r(bv_re[:]),
                             start=False, stop=True)
            nc.tensor.matmul(pi[:szk, :256], _r(ones_row[:, :szk]), _r(bv_im[:]),
                             start=False, stop=True)
            vr_v = pr[:szk, :256].rearrange("p (h d) -> p h d", h=4)
            vi_v = pi[:szk, :256].rearrange("p (h d) -> p h d", h=4)
            nc.vector.tensor_copy(vpk[:szk, kc, :, 0:64], vr_v)
            nc.vector.tensor_copy(vpk[:szk, kc, :, 64:128], vi_v)
    # kc4 (szk=64) packs head pairs along partitions in phase C: odd heads
    # read their v from rows 64:128
    nc.sync.dma_start(vpk[64:128, 4, 1, :], vpk[0:64, 4, 1, :])
    nc.gpsimd.dma_start(vpk[64:128, 4, 3, :], vpk[0:64, 4, 3, :])

    if DEBUG:
        for cj in range(4):
            nc.sync.dma_start(dbg["xnT"][0, 128 * cj:128 * (cj + 1), :],
                              xnTr[:, cj, :].bitcast(F32))
            nc.sync.dma_start(dbg["xnT"][1, 128 * cj:128 * (cj + 1), :],
                              xnTi[:, cj, :].bitcast(F32))
        nc.sync.dma_start(dbg["kA"][:], kA.bitcast(F32))
        nc.sync.dma_start(dbg["kB"][:], kB.bitcast(F32))

    bctx.close()

    # =====================================================================
    # Phase C: per q-chunk: q-proj -> scores -> softmax -> attn@v -> proj
    # =====================================================================
    cctx = contextlib.ExitStack()
    sm = cctx.enter_context(tc.tile_pool(name="sm", bufs=2))
    psC = cctx.enter_context(tc.tile_pool(name="psC", bufs=4, space="PSUM"))
    psOp = cctx.enter_context(tc.tile_pool(name="psOp", bufs=2, space="PSUM"))
    psDn = cctx.enter_context(tc.tile_pool(name="psDn", bufs=2, space="PSUM"))

    def emit_front(q0, nq, hp, qcs):
        """Scores + softmax numerators for head-pair hp; returns ebufs.

        kc 0-3: [szk, 2, nq] tiles (head i in dim 1).  kc 4 (szk=64):
        both heads packed in the partition dim (i0 rows 0:64, i1 64:128)
        so every elementwise op runs one [128, nq] pass instead of two.
        """
        sts = []
        for kc in range(4):
            k0, szk = K_CHUNKS[kc]
            s_t = sm.tile([128, 2, 512], F32, tag="s_t", bufs=4,
                          name=f"s{kc}")
            for i in range(2):
                h = 2 * hp + i
                qc = qcs[h]
                sre = psC.tile([128, 512], F32, tag="bank", name="sre")
                sim = psC.tile([128, 512], F32, tag="bank", name="sim")
                nc.tensor.matmul(sre[:szk, :nq], kA[:, h, k0:k0 + szk],
                                 qc[:, :nq], start=True, stop=True)
                nc.tensor.matmul(sim[:szk, :nq], kB[:, h, k0:k0 + szk],
                                 qc[:, :nq], start=True, stop=True)
                # s = sre^2 + sim^2 in f32
                nc.scalar.activation(s_t[:szk, i, :nq], sre[:szk, :nq],
                                     AF.Square)
                tmp = sm.tile([128, 512], F32, tag="tmp", bufs=2)
                if SQMODE == 0:
                    tmc = sm.tile([128, 512], F32, tag="tmc", bufs=2)
                    nc.vector.tensor_copy(tmc[:szk, :nq], sim[:szk, :nq])
                    nc.vector.tensor_mul(tmp[:szk, :nq], tmc[:szk, :nq],
                                         tmc[:szk, :nq])
                    nc.gpsimd.tensor_add(s_t[:szk, i, :nq], s_t[:szk, i, :nq],
                                         tmp[:szk, :nq])
                else:
                    nc.scalar.activation(tmp[:szk, :nq], sim[:szk, :nq],
                                         AF.Square)
                    nc.vector.tensor_add(s_t[:szk, i, :nq], s_t[:szk, i, :nq],
                                         tmp[:szk, :nq])
            if DEBUG and q0 == 0 and hp == 0 and kc < 2:
                nc.sync.dma_start(dbg["s"][kc, :szk, :, :nq], s_t[:szk, :, :nq])
            sts.append(s_t)
        # kc 4: head-pair packed [128, nq]
        s4 = sm.tile([128, 512], F32, tag="s4", bufs=3)
        sre4 = psC.tile([128, 512], F32, tag="bank", name="sre4")
        sim4 = psC.tile([128, 512], F32, tag="bank", name="sim4")
        for i in range(2):
            h = 2 * hp + i
            # i=0: cols [576:704) -> kc4 data lands in out rows 0:64;
            # i=1: cols [512:640) -> rows 64:128; zero cols pad the rest.
            c0 = 576 - 64 * i
            nc.tensor.matmul(sre4[:, :nq], kA[:, h, c0:c0 + 128],
                             qcs[h][:, :nq], start=i == 0, stop=i == 1)
            nc.tensor.matmul(sim4[:, :nq], kB[:, h, c0:c0 + 128],
                             qcs[h][:, :nq], start=i == 0, stop=i == 1)
        nc.scalar.activation(s4[:, :nq], sre4[:, :nq], AF.Square)
        tmp4 = sm.tile([128, 512], F32, tag="tmp", bufs=2)
        if SQMODE == 0:
            tmc4 = sm.tile([128, 512], F32, tag="tmc", bufs=2)
            nc.vector.tensor_copy(tmc4[:, :nq], sim4[:, :nq])
            nc.vector.tensor_mul(tmp4[:, :nq], tmc4[:, :nq], tmc4[:, :nq])
            nc.gpsimd.tensor_add(s4[:, :nq], s4[:, :nq], tmp4[:, :nq])
        else:
            nc.scalar.activation(tmp4[:, :nq], sim4[:, :nq], AF.Square)
            nc.vector.tensor_add(s4[:, :nq], s4[:, :nq], tmp4[:, :nq])
        # batched LUT runs: |a| = sqrt(s) (or exp(0.5 ln s)), then
        # ebuf = exp(|a| - 50) in bf16.  Softmax is shift-invariant so the
        # constant -50 cancels; it just keeps exp sums in range.
        if SQRTMODE == 0:
            for kc in range(4):
                szk = K_CHUNKS[kc][1]
                nc.scalar.activation(sts[kc][:szk, :, :nq],
                                     sts[kc][:szk, :, :nq], AF.Ln)
            nc.scalar.activation(s4[:, :nq], s4[:, :nq], AF.Ln)
            for kc in range(4):
                szk = K_CHUNKS[kc][1]
                nc.scalar.activation(sts[kc][:szk, :, :nq],
                                     sts[kc][:szk, :, :nq], AF.Exp, scale=0.5)
            nc.scalar.activation(s4[:, :nq], s4[:, :nq], AF.Exp, scale=0.5)
        else:
            for kc in range(4):
                szk = K_CHUNKS[kc][1]
                nc.scalar.activation(sts[kc][:szk, :, :nq],
                                     sts[kc][:szk, :, :nq], AF.Sqrt)
            nc.scalar.activation(s4[:, :nq], s4[:, :nq], AF.Sqrt)
        ebufs = []
        for kc in range(4):
            szk = K_CHUNKS[kc][1]
            ebuf = sm.tile([128, 2, 512], BF16, tag="ebuf", bufs=8,
                           name=f"eb{kc}")
            nc.scalar.activation(ebuf[:szk, :, :nq], sts[kc][:szk, :, :nq],
                                 AF.Exp, bias=nbias[:szk])
            ebufs.append(ebuf)
        eb4 = sm.tile([128, 512], BF16, tag="eb4", bufs=3)
        nc.scalar.activation(eb4[:, :nq], s4[:, :nq], AF.Exp, bias=nbias[:])
        ebufs.append(eb4)
        return ebufs

    def emit_back(q0, nq, hp, ebufs):
        """attn@v + denominators + normalize for head-pair hp."""
        ops = [psOp.tile([128, 512], F32, tag="op", name=f"op{i}")
               for i in range(2)]
        dnp = psDn.tile([128, 512], F32, tag="dn", name="dnp")
        for kc in range(4):
            k0, szk = K_CHUNKS[kc]
            ebuf = ebufs[kc]
            for i in range(2):
                h = 2 * hp + i
                nc.tensor.matmul(ops[i][:, :nq], vpk[:szk, kc, h, :],
                                 ebuf[:szk, i, :nq], start=kc == 0, stop=False)
                nc.tensor.matmul(dnp[32 * i:32 * i + 1, :nq], ones_col[:szk, :],
                                 ebuf[:szk, i, :nq], start=kc == 0, stop=False,
                                 tile_position=(0, 32 * i))
        eb4 = ebufs[4]
        for i in range(2):
            h = 2 * hp + i
            rs = slice(64 * i, 64 * (i + 1))
            mask = ones_h0 if i == 0 else ones_h1
            nc.tensor.matmul(ops[i][:, :nq], vpk[rs, 4, h, :],
                             eb4[rs, :nq], start=False, stop=True)
            nc.tensor.matmul(dnp[32 * i:32 * i + 1, :nq], mask[:, :],
                             eb4[:, :nq], start=False, stop=True,
                             tile_position=(0, 32 * i))
        dnt = sm.tile([1, 1024], F32, tag="dnt", bufs=2)
        dns = sm.tile([1, 1024], F32, tag="dns", bufs=2)
        dnr = sm.tile([1, 1024], F32R, tag="dnr", bufs=2)
        for i in range(2):
            c = slice(512 * i, 512 * i + nq)
            # ACT partition-shifts row 32i of PSUM into partition 0;
            # reciprocal_approx_fast is broken for base_partition != 0 on HW
            nc.scalar.copy(dnt[0:1, c], dnp[32 * i:32 * i + 1, :nq])
            if RECIPMODE == 0:
                nc.vector.reciprocal_approx_fast(dns[0:1, c], dnt[0:1, c])
                nc.vector.tensor_copy(dnr[0:1, c], dns[0:1, c])
            else:
                nc.scalar.activation(dns[0:1, c], dnt[0:1, c], AF.Ln)
                nc.scalar.activation(dnr[0:1, c], dns[0:1, c], AF.Exp,
                                     scale=-1.0)
        oris = []
        for i in range(2):
            osb = sm.tile([128, 512], F32, tag="osb", bufs=2, name=f"osb{i}")
            nc.scalar.copy(osb[:, :nq], ops[i][:, :nq])
            rbp = psC.tile([128, 512], F32, tag="bank", name="rbp")
            nc.tensor.matmul(rbp[:, :nq], _r(ones128[0:1, :]),
                             dnr[0:1, 512 * i:512 * i + nq],
                             start=True, stop=True)
            ori = sm.tile([128, 512], F16, tag="ori", bufs=5, name=f"ori{i}")
            nc.vector.tensor_mul(ori[:, :nq], osb[:, :nq], rbp[:, :nq])
            oris.append(ori)
        return oris

    def emit_proj(q0, nq, oris):
        """Partial output projection for one chunk; oris = [h0..h3]."""
        for cc in range(4):
            cs = slice(128 * cc, 128 * (cc + 1))
            pr = psC.tile([128, 512], F32, tag="bank", name="pjr")
            pi = psC.tile([128, 512], F32, tag="bank", name="pji")
            for h in range(4):
                st = h == 0
                sp = h == 3
                nc.tensor.matmul(pr[:, :nq], wps[:, 0, h, cs], oris[h][:, :nq],
                                 start=st, stop=sp)
                nc.tensor.matmul(pi[:, :nq], wps[:, 1, h, cs], oris[h][:, :nq],
                                 start=st, stop=sp)
            o1 = sm.tile([128, 512], F16, tag="o1", bufs=2)
            o2 = sm.tile([128, 512], F16, tag="o2", bufs=2)
            nc.vector.tensor_copy(o1[:, :nq], pr[:, :nq])
            nc.vector.tensor_copy(o2[:, :nq], pi[:, :nq])
            nc.gpsimd.dma_start(outT_d[0, cs, q0:q0 + nq], o1[:, :nq])
            nc.sync.dma_start(outT_d[1, cs, q0:q0 + nq], o2[:, :nq])

    # software pipeline, two half-steps deep: back(H[i-2]) is emitted after
    # front(H[i]) so its ebufs have had two full half-steps of ACT/DVE time;
    # the PE meanwhile streams sc(H[i]) + av(H[i-2]) + qp/proj with no stall.
    halves = [(q0, nq, hp) for (q0, nq) in Q_CHUNKS for hp in range(2)]
    qcs_by_q0 = dict(qcs_early)
    fronts = []
    ori_by_q0 = {}

    def _drain(idx):
        bq0, bnq, bhp, bebufs = fronts[idx]
        oris = emit_back(bq0, bnq, bhp, bebufs)
        ori_by_q0.setdefault(bq0, []).extend(oris)
        if bhp == 1:
            emit_proj(bq0, bnq, ori_by_q0.pop(bq0))

    for i, (q0, nq, hp) in enumerate(halves):
        ebufs = emit_front(q0, nq, hp, qcs_by_q0[q0])
        fronts.append((q0, nq, hp, ebufs))
        qi = i // 2
        if hp == 1 and qi + 2 < len(Q_CHUNKS):
            nq0, nnq = Q_CHUNKS[qi + 2]
            qcs_by_q0[nq0] = emit_qp(nq0, nnq, psC)
        if i >= 2:
            _drain(i - 2)
    _drain(len(halves) - 2)
    _drain(len(halves) - 1)

    cctx.close()
    ctx.close()


# =========================================================================
# Host side
# =========================================================================

def _f32(x):
    return np.ascontiguousarray(x, dtype=np.float32)


def _f16(x):
    return np.ascontiguousarray(np.asarray(x, dtype=np.float16))


def host_prep(x_re, x_im, Wq, Wkv, Wproj, bproj, sr_w, sr_b, gain, bias):
    x_re = np.asarray(x_re)
    x_im = np.asarray(x_im)
    Wq = np.asarray(Wq)
    Wkv = np.asarray(Wkv)
    Wproj = np.asarray(Wproj)
    sr_w = np.asarray(sr_w)
    sr_b = np.asarray(sr_b)
    gain = np.asarray(gain)
    bias = np.asarray(bias)

    Wkv_eff = gain[:, None] * Wkv
    bkv_full = bias @ Wkv
    Wc = sr_w.transpose(2, 3, 1, 0).reshape(4 * C, C)

    def planes3f(w):
        return np.stack([_f32(w.real), _f32(w.imag), _f32(-w.imag)])

    in_maps = []
    for core in range(8):
        b, g = core // 2, core % 2
        cols = slice(256 * g, 256 * (g + 1))
        wk_c = Wkv_eff[:, :C][:, cols] * SCALE
        wv_c = Wkv_eff[:, C:][:, cols]
        bk_c = bkv_full[:C][cols] * SCALE
        bv_c = bkv_full[C:][cols]
        xs_c = np.stack([x_re[b].T, x_im[b].T])  # [2, C, N]
        xsp = xs_c.reshape(2, C, HR, 2, HR, 2)
        xP = np.stack([xsp[:, :, :, p, :, q].reshape(2, C, NK)
                       for p in range(2) for q in range(2)], axis=1)
        xP = xP[:, :, :, 288 * g:288 * (g + 1)]  # this core's conv rows
        # wpA_h = [Wp_re rows of head h ; -Wp_im rows], wpB_h = [im ; re]
        wp_blk = Wproj[256 * g:256 * (g + 1), :]  # [256, C] complex
        wpab = np.zeros((2, 4, 128, C), np.float16)
        for h in range(4):
            rows = wp_blk[64 * h:64 * (h + 1), :]
            wpab[0, h, 0:64] = _f16(rows.real)
            wpab[0, h, 64:128] = _f16(-rows.imag)
            wpab[1, h, 0:64] = _f16(rows.imag)
            wpab[1, h, 64:128] = _f16(rows.real)
        m = {
            "xT": _f32(xs_c),
            "xP": _f32(xP.reshape(2, 4 * C, 288)),
            "wc": planes3f(Wc),
            "srb": np.stack([_f32(sr_b.real), _f32(sr_b.imag)]),
            "ones": np.ones((1, 512), np.float32),
            "wq": planes3f(Wq[:, cols]),
            "wk": planes3f(wk_c),
            "wv": planes3f(wv_c),
            "wpab": wpab,
            "bkv": np.stack([
                np.stack([_f32(bk_c.real), _f32(bv_c.real)]),
                np.stack([_f32(bk_c.imag), _f32(bv_c.imag)]),
            ]),
        }
        in_maps.append(m)
    return in_maps


_NC_CACHE = None


def _get_nc():
    global _NC_CACHE
    if _NC_CACHE is None:
        _NC_CACHE = build_nc()
    return _NC_CACHE


def kernel(x_re, x_im, Wq, Wkv, Wproj, bproj, sr_w, sr_b, gain, bias, H, W):
    from concourse.bass_utils import run_bass_kernel_spmd

    nc = _get_nc()
    in_maps = host_prep(x_re, x_im, Wq, Wkv, Wproj, bproj, sr_w, sr_b, gain, bias)
    res = run_bass_kernel_spmd(nc, in_maps, list(range(8)))
    bproj = np.asarray(bproj)
    out = np.zeros((B, N, C), dtype=np.complex64)
    for b in range(B):
        p0 = res.results[2 * b]["outT"].astype(np.float32)
        p1 = res.results[2 * b + 1]["outT"].astype(np.float32)
        acc = (p0[0] + p1[0]).T + 1j * (p0[1] + p1[1]).T
        out[b] = acc + bproj[None, :]
    return out
